# revision 13
# baseline (speedup 1.0000x reference)
"""GCN 2-layer + mean-pool + log_softmax kernel for 8x TRN2 cores.

v2 design (vs v1 baseline at 309us):
  - dinv[src] folded into xT on host (no on-chip y1 scaling)
  - replication via AllGather with flat-lowered APs into a dense row table
    (y1cc [12608 rows x 32B] per core, 64 embedded zero-pad rows), then one
    strided re-layout DMA into the 256B-stride gather table
  - gathers fetch int32 elements (8 x i32 = 32B for L1, 2 x i32 = 8B for L2)
  - scatter-add via fp8 DoubleRow matmuls (2 chunks / instruction)
  - per-block finalize relu(agg*dinv) on the Activation engine (scale AP)
  - stage C (h1 -> y2) per block: PE transpose + W2 matmul + DVE scale
  - host-precomputed one-hot S matrices (fixed + spill) uploaded as fp8
"""
import sys
for _p in ("/opt/trn_rl_repo", "/root/.axon_site/_ro/trn_rl_repo"):
    if _p not in sys.path:
        sys.path.append(_p)
import inspect
import numpy as np
import ml_dtypes

import concourse.bass as bass
import concourse.bacc as bacc
import concourse.mybir as mybir
import concourse.tile as tile

BF = ml_dtypes.bfloat16
F8 = ml_dtypes.float8_e4m3
P = 128
NCORES = 8
IN_CH = 256
HID = 32
OUT = 8
NG = 64
NGROUPS = 4
KSLOT = 4              # fixed slots per (lane, group)
LPC = P // KSLOT       # lanes per regular chunk
ZR = 64                # zero-pad rows embedded at the end of each core slab
SENT = 1000.0


def _install_patched_gather():
    if hasattr(bass.BassGpSimd, "dma_gather_p"):
        return True
    src = inspect.getsource(bass.BassGpSimd.dma_gather)
    src = src.replace(
        "elem_size_bytes > 0 and elem_size_bytes % 256 == 0",
        "elem_size_bytes > 0 and elem_size_bytes % 8 == 0")
    src = "def dma_gather_p" + src[src.index("("):]
    ns = dict(bass.__dict__)
    exec(compile(src, "dma_gather_p", "exec"), ns)
    bass.BassGpSimd.dma_gather_p = ns["dma_gather_p"]
    return True


def _collective_raw(eng, kind, op, in_ap, out_ap, replica_groups):
    """collective_compute with flat [[1,N],[1,1]] lowered APs (opt=False)."""
    from concourse.replica_groups import filter_and_check_groups
    eng.bass.has_collectives = True
    rg = filter_and_check_groups(eng.bass.num_devices, replica_groups)
    return eng.add_instruction(
        mybir.InstCollectiveCompute(
            name=f"I-{eng.bass.next_id()}",
            kind=kind, op=op, replica_groups=rg,
            ins=[eng.lower_ap(in_ap.rearrange("a b -> (a b)").unsqueeze(1),
                              opt=False)],
            outs=[eng.lower_ap(out_ap.rearrange("a b -> (a b)").unsqueeze(1),
                               opt=False)],
            unique_tensors="No", cc_dim="Partition"))


class Plan:
    """Uniform (core-independent) chunk schedule."""
    def __init__(self, bpc, nch_bg, call_plan, nsp_col_of, maxch):
        self.bpc = bpc
        self.npc = bpc * P
        self.slabr = self.npc + ZR
        self.winr = 2 * self.slabr
        self.trows = NCORES * self.slabr
        self.nch_bg = nch_bg            # [bpc][NGROUPS]
        self.call_plan = call_plan      # per group: list of (q_start, nchunks)
        self.nsp_col_of = nsp_col_of    # (b, g) -> first spill col
        self.nsp_cols = max(nsp_col_of.values(), default=-1) + 1 if nsp_col_of else 0
        self.maxch = maxch
        self.prefix_g = np.zeros((bpc + 1, NGROUPS), np.int64)
        for b in range(bpc):
            self.prefix_g[b + 1] = self.prefix_g[b] + nch_bg[b]
        self.call_col = {}
        col = 0
        for g in range(NGROUPS):
            lst = []
            for (q0, nch) in call_plan[g]:
                lst.append(col)
                col += nch * P // 16
            self.call_col[g] = lst
        self.idx_cols = col


def host_prep(x, edge_index, batch, maxch=96):
    N = x.shape[0]
    src = np.asarray(edge_index[0], np.int64)
    dst = np.asarray(edge_index[1], np.int64)
    batch = np.asarray(batch, np.int64)

    npc = -(-N // (NCORES * P)) * P
    bpc = npc // P
    slabr = npc + ZR
    winr = 2 * slabr
    assert winr <= 32768

    core = dst // npc
    blk = (dst % npc) // P
    lane = dst % P

    # block -> slot permutation balancing in-edge count across cores
    cnt_cb = np.bincount(core * bpc + blk, minlength=NCORES * bpc).reshape(NCORES, bpc)
    perm = np.argsort(-cnt_cb, axis=1, kind="stable")
    invperm = np.argsort(perm, axis=1)
    slot = invperm[core, blk]

    core_s = src // npc
    slot_src = invperm[core_s, (src % npc) // P]
    lane_src = src % P
    grp = core_s // 2
    rel = ((core_s % 2) * slabr + slot_src * P + lane_src).astype(np.int64)
    PAD = npc  # first zero row of the even slab, valid in every window

    # sort edges by (core, slot, group, lane)
    key = ((core * bpc + slot) * NGROUPS + grp) * P + lane
    order = np.argsort(key, kind="stable")
    key_s = key[order]
    rel_s = rel[order]
    nkeys = NCORES * bpc * NGROUPS * P
    cnt4 = np.bincount(key_s, minlength=nkeys)
    seg_start = np.concatenate([[0], np.cumsum(cnt4)])[:-1]
    ordinal = np.arange(len(key_s)) - seg_start[key_s]

    cnt4r = cnt4.reshape(NCORES, bpc, NGROUPS, P)
    spill = np.maximum(cnt4r - KSLOT, 0)
    spill_bg_c = spill.sum(axis=3)
    nspill_bg = -(-spill_bg_c.max(axis=0) // P)
    nch_bg = (KSLOT + nspill_bg).astype(np.int64)

    nsp_col_of = {}
    col = 0
    for b in range(bpc):
        for g in range(NGROUPS):
            if nspill_bg[b, g]:
                nsp_col_of[(b, g)] = col
                col += int(nspill_bg[b, g])
    nsp_cols = col

    chunks_g = (KSLOT + nspill_bg).sum(axis=0)
    call_plan = []
    for g in range(NGROUPS):
        ncg = int(chunks_g[g])
        calls, q = [], 0
        while q < ncg:
            n = min(maxch, ncg - q)
            calls.append((q, n))
            q += n
        call_plan.append(calls)

    plan = Plan(bpc, nch_bg.tolist(), call_plan, nsp_col_of, maxch)
    prefix_g = plan.prefix_g

    deg_full = np.bincount(dst, minlength=N).astype(np.float32) + 1.0
    dinv_full = 1.0 / np.sqrt(deg_full)

    core_e = key_s // (bpc * NGROUPS * P)
    rem = key_s % (bpc * NGROUPS * P)
    slot_e = rem // (NGROUPS * P)
    g_e = (rem // P) % NGROUPS
    lane_e = rem % P

    # fixed one-hot S (shared by all cores)
    s_ar = np.arange(P)
    sfx = np.zeros((P, KSLOT * P), F8)
    for r in range(KSLOT):
        sfx[s_ar, r * P + r * LPC + s_ar // KSLOT] = 1.0
    ident8 = np.eye(P, dtype=np.float32).astype(F8)
    identbf = np.eye(P, dtype=np.float32).astype(BF)

    per_core = []
    for c in range(NCORES):
        m = core_e == c
        sl, gg, ln, o, rr = slot_e[m], g_e[m], lane_e[m], ordinal[m], rel_s[m]

        idx_groups = [np.full((int(chunks_g[g]) * P,), PAD, np.int16)
                      for g in range(NGROUPS)]
        dl = np.full((max(nsp_cols, 1), P), -1, np.int64)

        regm = o < KSLOT
        q = prefix_g[sl[regm], gg[regm]] + ln[regm] // LPC
        pos = q * P + (ln[regm] % LPC) * KSLOT + o[regm]
        for g in range(NGROUPS):
            gm = gg[regm] == g
            idx_groups[g][pos[gm]] = rr[regm][gm].astype(np.int16)

        spm = ~regm
        if spm.any():
            key2 = sl[spm] * NGROUPS + gg[spm]
            cnt2 = np.bincount(key2, minlength=bpc * NGROUPS)
            st2 = np.concatenate([[0], np.cumsum(cnt2)])[:-1]
            so = np.arange(len(key2)) - st2[key2]
            qsp = prefix_g[sl[spm], gg[spm]] + KSLOT + so // P
            pos = qsp * P + so % P
            for g in range(NGROUPS):
                gm = gg[spm] == g
                idx_groups[g][pos[gm]] = rr[spm][gm].astype(np.int16)
            colbase = np.array([nsp_col_of.get((b, g), 0)
                                for b in range(bpc) for g in range(NGROUPS)],
                               np.int64).reshape(bpc, NGROUPS)
            cols = colbase[sl[spm], gg[spm]] + so // P
            dl[cols, so % P] = ln[spm]

        # spill one-hot S: [P rows, nsp_cols * P dst]
        ssp = np.zeros((P, max(nsp_cols, 1) * P), F8)
        qq, rrow = np.nonzero(dl >= 0)
        ssp[rrow, qq * P + dl[qq, rrow]] = 1.0

        cols_list = []
        for g in range(NGROUPS):
            arr = idx_groups[g]
            for (q0, nch) in call_plan[g]:
                seg = arr[q0 * P:(q0 + nch) * P]
                nid = nch * P
                w = np.zeros((16, nid // 16), np.int16)
                ii = np.arange(nid)
                w[ii % 16, ii // 16] = seg
                cols_list.append(np.tile(w, (8, 1)))
        idx_in = np.concatenate(cols_list, axis=1) if cols_list else np.zeros((P, 1), np.int16)

        # per-core node data (slot-permuted)
        nbase = c * npc
        dinvc = np.zeros((npc,), np.float32)
        hi = min(nbase + npc, N)
        if hi > nbase:
            dinvc[:hi - nbase] = dinv_full[nbase:hi]
        bl = np.full((npc,), -1, np.int64)
        if hi > nbase:
            bl[:hi - nbase] = batch[nbase:hi]
        dinvc = dinvc.reshape(bpc, P)[perm[c]].reshape(npc)
        bl = bl.reshape(bpc, P)[perm[c]].reshape(npc)
        dinv_t = dinvc.reshape(bpc, P).T.copy()               # [P, bpc]
        dinvrep8 = np.repeat(dinv_t[:, :, None], OUT, axis=2).reshape(P, bpc * OUT)
        # pooling one-hot: [P, bpc*NG]
        blm = bl.reshape(bpc, P).T                            # [P, bpc]
        sb = np.zeros((P, bpc, NG), np.float32)
        pi, bi = np.nonzero(blm >= 0)
        sb[pi, bi, blm[pi, bi]] = 1.0
        sb = sb.reshape(P, bpc * NG).astype(BF)

        per_core.append(dict(idx=idx_in, ssp=ssp, dinv=dinv_t,
                             dinv_l1=(dinv_t / 4.0).astype(np.float32),
                             dinvrep8=dinvrep8.astype(np.float32), sb=sb))

    cnts = np.bincount(batch[batch >= 0], minlength=NG).astype(np.float32)

    # xT with alpha*dinv folded, slot-permuted columns
    ALPHA = 4.0
    Np = npc * NCORES
    xs = np.asarray(x, np.float32) * (ALPHA * dinv_full)[:, None]
    xT = np.zeros((IN_CH, Np), F8)
    xT[:, :N] = xs.T.astype(F8)
    colperm = np.empty((Np,), np.int64)
    for c in range(NCORES):
        base = c * npc
        colperm[base:base + npc] = base + (perm[c][:, None] * P +
                                           np.arange(P)[None, :]).reshape(-1)
    xT = xT[:, colperm]
    return plan, per_core, cnts, xT, sfx, ident8, identbf


def build(plan: Plan, with_b1, with_b2):
    _install_patched_gather()
    nc = bacc.Bacc("TRN2", target_bir_lowering=False, debug=False,
                   num_swdge_queues=1, dynamic_dma_scratch_size=65536)
    f32, bf16, i16, i32 = (mybir.dt.float32, mybir.dt.bfloat16,
                           mybir.dt.int16, mybir.dt.int32)
    fp8 = mybir.dt.float8e4
    A = mybir.AluOpType
    AF = mybir.ActivationFunctionType
    DR = mybir.MatmulPerfMode.DoubleRow
    npc, bpc = plan.npc, plan.bpc
    SLABR, WINR, TROWS = plan.slabr, plan.winr, plan.trows
    NSP = max(plan.nsp_cols, 1)

    xT = nc.dram_tensor("xT", [IN_CH, npc], fp8, kind="ExternalInput")
    idx = nc.dram_tensor("idx", [P, plan.idx_cols], i16, kind="ExternalInput")
    sfx = nc.dram_tensor("sfx", [P, KSLOT * P], fp8, kind="ExternalInput")
    ssp = nc.dram_tensor("ssp", [P, NSP * P], fp8, kind="ExternalInput")
    ident8 = nc.dram_tensor("ident8", [P, P], fp8, kind="ExternalInput")
    identbf = nc.dram_tensor("identbf", [P, P], bf16, kind="ExternalInput")
    W1 = nc.dram_tensor("W1", [P, 2 * HID], f32, kind="ExternalInput")
    W2 = nc.dram_tensor("W2", [HID, OUT], f32, kind="ExternalInput")
    b1 = nc.dram_tensor("b1", [1, HID], f32, kind="ExternalInput")
    b2 = nc.dram_tensor("b2", [1, OUT], f32, kind="ExternalInput")
    dinv = nc.dram_tensor("dinv", [P, bpc], f32, kind="ExternalInput")
    dinv_l1 = nc.dram_tensor("dinv_l1", [P, bpc], f32, kind="ExternalInput")
    w2rep = nc.dram_tensor("w2rep", [P, OUT], f32, kind="ExternalInput")
    dinvrep8 = nc.dram_tensor("dinvrep8", [P, bpc * OUT], f32, kind="ExternalInput")
    sb = nc.dram_tensor("sb", [P, bpc * NG], bf16, kind="ExternalInput")
    out = nc.dram_tensor("out", [NG, OUT], f32, kind="ExternalOutput")

    y1cc = nc.dram_tensor("y1cc", [SLABR, 8], i32, kind="Internal")
    y1win = nc.dram_tensor("y1win", [TROWS, 8], i32, kind="Internal")
    y1tab = nc.dram_tensor("y1tab", [TROWS, 64], i32, kind="Internal")
    y2cc = nc.dram_tensor("y2cc", [SLABR, 2], i32, kind="Internal")
    y2win = nc.dram_tensor("y2win", [TROWS, 2], i32, kind="Internal")
    y2tab = nc.dram_tensor("y2tab", [TROWS, 64], i32, kind="Internal")

    with tile.TileContext(nc) as tc:
        with tc.tile_pool(name="const", bufs=1) as cpool, \
             tc.tile_pool(name="persist", bufs=1) as pers, \
             tc.tile_pool(name="g0", bufs=2) as gp0, \
             tc.tile_pool(name="g1", bufs=2) as gp1, \
             tc.tile_pool(name="g2", bufs=2) as gp2, \
             tc.tile_pool(name="g3", bufs=2) as gp3, \
             tc.tile_pool(name="spool", bufs=8) as spool, \
             tc.tile_pool(name="psB", bufs=2, space="PSUM") as psB, \
             tc.tile_pool(name="psT", bufs=2, space="PSUM") as psT, \
             tc.tile_pool(name="ps2p", bufs=1, space="PSUM") as ps2p, \
             tc.tile_pool(name="psE", bufs=1, space="PSUM") as psE:
            gpools = [gp0, gp1, gp2, gp3]

            # ---- small constants on the scalar queue ----
            w1f = cpool.tile([P, 2 * HID], f32)
            nc.scalar.dma_start(out=w1f[:], in_=W1[:, :])
            w1t = cpool.tile([P, 2 * HID], bf16)
            nc.vector.tensor_copy(out=w1t[:], in_=w1f[:])
            w2f = cpool.tile([HID, OUT], f32)
            nc.scalar.dma_start(out=w2f[:], in_=W2[:, :])
            w2t = cpool.tile([HID, OUT], bf16)
            nc.vector.tensor_copy(out=w2t[:], in_=w2f[:])
            dinv_t = cpool.tile([P, bpc], f32)
            nc.scalar.dma_start(out=dinv_t[:], in_=dinv[:, :])
            dinvl1_t = cpool.tile([P, bpc], f32)
            nc.scalar.dma_start(out=dinvl1_t[:], in_=dinv_l1[:, :])
            w2rf = cpool.tile([P, OUT], f32)
            nc.scalar.dma_start(out=w2rf[:], in_=w2rep[:, :])
            w2rept = cpool.tile([P, OUT], bf16)
            nc.vector.tensor_copy(out=w2rept[:], in_=w2rf[:])
            dr8 = cpool.tile([P, bpc * OUT], f32)
            nc.scalar.dma_start(out=dr8[:], in_=dinvrep8[:, :])
            sfx_t = cpool.tile([P, KSLOT * P], fp8)
            nc.scalar.dma_start(out=sfx_t[:], in_=sfx[:, :])
            id8_t = cpool.tile([P, P], fp8)
            nc.scalar.dma_start(out=id8_t[:], in_=ident8[:, :])
            idbf_t = cpool.tile([P, P], bf16)
            nc.scalar.dma_start(out=idbf_t[:], in_=identbf[:, :])
            if with_b1 or with_b2:
                b1t = cpool.tile([1, HID], f32)
                nc.scalar.dma_start(out=b1t[:], in_=b1[:, :])
                b2t = cpool.tile([1, OUT], f32)
                nc.scalar.dma_start(out=b2t[:], in_=b2[:, :])
                ones_col = cpool.tile([1, P], f32)
                nc.gpsimd.memset(ones_col[:], 1.0)
                b1b_ps = psB.tile([P, HID], f32, name="b1b_ps")
                nc.tensor.matmul(out=b1b_ps[:], lhsT=ones_col[:], rhs=b1t[:],
                                 start=True, stop=True)
                b1b = cpool.tile([P, HID], f32)
                nc.vector.tensor_copy(out=b1b[:], in_=b1b_ps[:])
                b2b_ps = psB.tile([P, OUT], f32, name="b2b_ps")
                nc.tensor.matmul(out=b2b_ps[:], lhsT=ones_col[:], rhs=b2t[:],
                                 start=True, stop=True)
                b2b = cpool.tile([P, OUT], f32)
                nc.vector.tensor_copy(out=b2b[:], in_=b2b_ps[:])

            # idx + pooling one-hots (scalar queue, hidden under AG1)
            idx_t = pers.tile([P, plan.idx_cols], i16)
            nc.scalar.dma_start(out=idx_t[:], in_=idx[:, :])
            sb_t = pers.tile([P, bpc * NG], bf16)
            nc.scalar.dma_start(out=sb_t[:], in_=sb[:, :])

            ssp_t = pers.tile([P, NSP * P], fp8)

            # persistent activations
            y1_sh = pers.tile([P, bpc * HID], fp8)
            h1_sh = pers.tile([P, bpc * HID], bf16)
            y2_sh = pers.tile([P, bpc * OUT], fp8)
            h2_sh = pers.tile([P, bpc * OUT], bf16)

            # zero-pad rows of y1cc / y2cc (gpsimd queue)
            z32 = cpool.tile([P, 16], i32)
            nc.gpsimd.memset(z32[:], 0)
            nc.gpsimd.dma_start(
                out=y1cc[npc:SLABR, :].rearrange("(a b) c -> a (b c)", a=ZR),
                in_=z32[0:ZR, 0:8])
            nc.gpsimd.dma_start(
                out=y2cc[npc:SLABR, :].rearrange("(a b) c -> a (b c)", a=ZR),
                in_=z32[0:ZR, 0:2])

            # ---- stage A: y1 = (x*dinv) @ W1 ----
            SLAB = 8
            nslab = -(-bpc // SLAB)
            sbA_ctx = tc.tile_pool(name="sbA", bufs=2)
            sbA = sbA_ctx.__enter__()
            psA_ctx = tc.tile_pool(name="psA", bufs=2, space="PSUM")
            psA = psA_ctx.__enter__()

            def write_y1cc(piece):
                b0 = 0 if piece == 0 else 7 * SLAB
                b1_ = 7 * SLAB if piece == 0 else bpc
                nc.sync.dma_start(
                    out=y1cc[b0 * P:b1_ * P, :].rearrange("(b p) c -> p b c", p=P),
                    in_=y1_sh[:, b0 * HID:b1_ * HID].bitcast(i32).rearrange(
                        "p (b c) -> p b c", c=8))

            for s in range(nslab):
                s0 = s * SLAB
                sbk = min(SLAB, bpc - s0)
                xt0 = sbA.tile([P, SLAB * P], fp8, tag="xt0")
                xt1 = sbA.tile([P, SLAB * P], fp8, tag="xt1")
                nc.sync.dma_start(out=xt0[:, :sbk * P], in_=xT[0:P, s0 * P:(s0 + sbk) * P])
                nc.gpsimd.dma_start(out=xt1[:, :sbk * P], in_=xT[P:2 * P, s0 * P:(s0 + sbk) * P])
                ps = psA.tile([P, SLAB * HID], f32, tag="pst")
                for j in range(sbk):
                    nc.tensor.matmul(out=ps[:, j * HID:(j + 1) * HID],
                                     lhsT=xt0[:, j * P:(j + 1) * P],
                                     rhs=w1t[:, 0:HID], start=True, stop=False)
                    nc.tensor.matmul(out=ps[:, j * HID:(j + 1) * HID],
                                     lhsT=xt1[:, j * P:(j + 1) * P],
                                     rhs=w1t[:, HID:2 * HID], start=False, stop=True)
                nc.vector.tensor_copy(out=y1_sh[:, s0 * HID:(s0 + sbk) * HID],
                                      in_=ps[:, :sbk * HID])
                if s == 6:
                    write_y1cc(0)
            write_y1cc(1)
            psA_ctx.__exit__(None, None, None)
            sbA_ctx.__exit__(None, None, None)

            # spill one-hots (sync queue, hidden under AG1)
            half = (NSP // 2) * P
            if half:
                nc.sync.dma_start(out=ssp_t[:, 0:half], in_=ssp[:, 0:half])
            nc.sync.dma_start(out=ssp_t[:, half:], in_=ssp[:, half:])

            # ---- replicate layer-1 table ----
            _collective_raw(nc.gpsimd, "AllGather", A.bypass,
                            y1cc[:, :], y1win[:, :], [list(range(NCORES))])
            TH = TROWS // 2
            nc.scalar.dma_start(out=y1tab[0:TH, 0:8], in_=y1win[0:TH, :])
            nc.scalar.dma_start(out=y1tab[TH:TROWS, 0:8], in_=y1win[TH:TROWS, :])

            # ---- aggregation ----
            def agg_layer(ytab, ysh, hsh, C, EL, gtag, with_b, bb, post_block,
                          dscale):
                gstate = [dict(call=-1, tile=None) for _ in range(NGROUPS)]

                def ensure_call(g, q):
                    cidx = 0
                    for i, (q0, nch) in enumerate(plan.call_plan[g]):
                        if q0 <= q < q0 + nch:
                            cidx = i
                            break
                    st = gstate[g]
                    if st["call"] == cidx:
                        return st["tile"], plan.call_plan[g][cidx][0], cidx
                    (q0, nch) = plan.call_plan[g][cidx]
                    gt = gpools[g].tile([P, plan.maxch * EL], i32,
                                        tag=f"{gtag}{g}", name=f"{gtag}t{g}")
                    col = plan.call_col[g][cidx]
                    nid = nch * P
                    nc.gpsimd.dma_gather_p(
                        out_ap=gt[:, :nch * EL].rearrange("p (k c) -> p k c", c=EL),
                        in_ap=ytab[g * WINR:(g + 1) * WINR, 0:EL],
                        idxs_ap=idx_t[:, col:col + nid // 16],
                        num_idxs=nid, num_idxs_reg=nid,
                        elem_size=EL, elem_step=64, single_packet=False)
                    st["call"] = cidx
                    st["tile"] = gt
                    return gt, q0, cidx

                qcol = 0
                for b in range(bpc):
                    ps = psB.tile([P, C], f32, name="psB_t")
                    nchunks = sum(plan.nch_bg[b])
                    nc.tensor.matmul(out=ps[:], lhsT=id8_t[:],
                                     rhs=ysh[:, b * C:(b + 1) * C],
                                     start=True, stop=False)
                    done = 0
                    for g in range(NGROUPS):
                        nch_b = plan.nch_bg[b][g]
                        j = 0
                        while j < nch_b:
                            q = int(plan.prefix_g[b, g]) + j
                            gt, q0, cidx = ensure_call(g, q)
                            sl = q - q0
                            # pairable: next chunk exists, same call, same kind
                            import os as _os
                            same_kind = (j + 1 < nch_b and
                                         (j + 1 < KSLOT) == (j < KSLOT) and
                                         not int(_os.environ.get("GNN_NODR", "0")))
                            same_call = (q + 1 < plan.call_plan[g][cidx][0] +
                                         plan.call_plan[g][cidx][1])
                            if same_kind and same_call:
                                if j < KSLOT:
                                    S_ap = sfx_t[:, j * P:(j + 2) * P]
                                else:
                                    S_ap = ssp_t[:, qcol * P:(qcol + 2) * P]
                                    qcol += 2
                                done += 2
                                nc.tensor.matmul(
                                    out=ps[:],
                                    lhsT=S_ap.rearrange("p (two m) -> p two m", two=2),
                                    rhs=gt[:, sl * EL:(sl + 2) * EL].bitcast(fp8)
                                          .rearrange("p (two c) -> p two c", two=2),
                                    start=False, stop=(done == nchunks),
                                    perf_mode=DR)
                                j += 2
                            else:
                                if j < KSLOT:
                                    S_ap = sfx_t[:, j * P:(j + 1) * P]
                                else:
                                    S_ap = ssp_t[:, qcol * P:(qcol + 1) * P]
                                    qcol += 1
                                done += 1
                                nc.tensor.matmul(
                                    out=ps[:], lhsT=S_ap,
                                    rhs=gt[:, sl * EL:(sl + 1) * EL].bitcast(fp8)[:, 0:C],
                                    start=False, stop=(done == nchunks))
                                j += 1
                    dv = dscale[:, b:b + 1]
                    if with_b:
                        t1 = spool.tile([P, C], f32, name="t1_t")
                        nc.vector.tensor_scalar(
                            out=t1[:], in0=ps[:], scalar1=dv,
                            scalar2=None, op0=A.mult)
                        t2 = spool.tile([P, C], f32, name="t2_t")
                        nc.vector.tensor_tensor(out=t2[:], in0=t1[:],
                                                in1=bb[:, :C], op=A.add)
                        nc.scalar.activation(out=hsh[:, b * C:(b + 1) * C],
                                             in_=t2[:], func=AF.Relu)
                    elif b % 2 == 0 and False:
                        nc.scalar.activation(out=hsh[:, b * C:(b + 1) * C],
                                             in_=ps[:], func=AF.Relu,
                                             scale=dv)
                    else:
                        nc.vector.tensor_scalar(
                            out=hsh[:, b * C:(b + 1) * C], in0=ps[:],
                            scalar1=dv, scalar2=0.0, op0=A.mult, op1=A.max)
                    if post_block is not None:
                        post_block(b)

            # stage C per block: transpose h1 block, @W2, scale -> y2_sh
            def stage_c(b):
                tp = psT.tile([HID, P], bf16, name="psT_t", tag="tp")
                nc.tensor.transpose(out=tp[:], in_=h1_sh[:, b * HID:(b + 1) * HID],
                                    identity=idbf_t[:])
                h1Tb = spool.tile([HID, P], bf16, name="h1Tb_t")
                nc.vector.tensor_copy(out=h1Tb[:], in_=tp[:])
                ps2 = ps2p.tile([P, OUT], f32, name="ps2_t")
                nc.tensor.matmul(out=ps2[:], lhsT=h1Tb[:],
                                 rhs=w2t[:], start=True, stop=True)
                nc.vector.tensor_tensor(
                    out=y2_sh[:, b * OUT:(b + 1) * OUT], in0=ps2[:],
                    in1=dr8[:, b * OUT:(b + 1) * OUT], op=A.mult)
                if b == 7 * SLAB - 1:
                    write_y2cc(0)

            def write_y2cc(piece):
                b0 = 0 if piece == 0 else 7 * SLAB
                b1_ = 7 * SLAB if piece == 0 else bpc
                nc.sync.dma_start(
                    out=y2cc[b0 * P:b1_ * P, :].rearrange("(b p) c -> p b c", p=P),
                    in_=y2_sh[:, b0 * OUT:b1_ * OUT].bitcast(i32).rearrange(
                        "p (b c) -> p b c", c=2))

            agg_layer(y1tab, y1_sh, h1_sh, HID, 8, "ga", with_b1,
                      b1b if with_b1 else None, stage_c, dinvl1_t)
            write_y2cc(1)

            # ---- replicate layer-2 table ----
            _collective_raw(nc.gpsimd, "AllGather", A.bypass,
                            y2cc[:, :], y2win[:, :], [list(range(NCORES))])
            nc.scalar.dma_start(out=y2tab[0:TH, 0:2], in_=y2win[0:TH, :])
            nc.scalar.dma_start(out=y2tab[TH:TROWS, 0:2], in_=y2win[TH:TROWS, :])

            # layer 2 + pooling per block
            pp = psE.tile([NG, OUT], f32)

            def pool_block(b):
                nc.tensor.matmul(out=pp[:], lhsT=sb_t[:, b * NG:(b + 1) * NG],
                                 rhs=h2_sh[:, b * OUT:(b + 1) * OUT],
                                 start=(b == 0), stop=(b == bpc - 1))

            agg_layer(y2tab, y2_sh, h2_sh, OUT, 2, "gb", with_b2,
                      b2b if with_b2 else None, pool_block, dinv_t)

            sums = cpool.tile([NG, OUT], f32)
            nc.vector.tensor_copy(out=sums[:], in_=pp[:])
            nc.sync.dma_start(out=out[:, :], in_=sums[:])

    nc.compile()
    return nc


def prep_program(x, edge_index, batch, W1, b1, W2, b2):
    """Build the compiled program + per-core input maps + pooling counts."""
    plan, per_core, cnts, xT, sfx, ident8, identbf = host_prep(x, edge_index, batch)
    with_b1 = bool(np.any(np.asarray(b1)))
    with_b2 = bool(np.any(np.asarray(b2)))
    nc = build(plan, with_b1, with_b2)
    W1a = np.asarray(W1, np.float32)
    w1h = np.concatenate([W1a[0:P, :], W1a[P:2 * P, :]], axis=1)  # [P, 2*HID]
    W2a = np.asarray(W2, np.float32)
    w2r = np.zeros((P, OUT), np.float32)
    for j in range(4):
        w2r[j * HID:(j + 1) * HID] = W2a
    in_maps = []
    for c in range(NCORES):
        pc = per_core[c]
        m = dict(
            xT=np.ascontiguousarray(xT[:, c * plan.npc:(c + 1) * plan.npc]),
            idx=pc["idx"], sfx=sfx, ssp=pc["ssp"], ident8=ident8,
            identbf=identbf, W1=w1h, W2=np.asarray(W2, np.float32),
            b1=np.asarray(b1, np.float32).reshape(1, HID),
            b2=np.asarray(b2, np.float32).reshape(1, OUT),
            dinv=pc["dinv"], dinv_l1=pc["dinv_l1"],
            dinvrep8=pc["dinvrep8"], sb=pc["sb"], w2rep=w2r)
        in_maps.append(m)
    return nc, in_maps, cnts


def run_gnn(x, edge_index, batch, W1, b1, W2, b2):
    from concourse.bass_utils import run_bass_kernel_spmd
    nc, in_maps, cnts = prep_program(x, edge_index, batch, W1, b1, W2, b2)
    res = run_bass_kernel_spmd(nc, in_maps, core_ids=list(range(NCORES)))
    total = np.zeros((NG, OUT), np.float64)
    for c in range(NCORES):
        total += np.asarray(res.results[c]["out"], np.float64)
    pooled = total / np.maximum(cnts, 1.0)[:, None]
    z = pooled - pooled.max(axis=1, keepdims=True)
    ls = z - np.log(np.exp(z).sum(axis=1, keepdims=True))
    return ls.astype(np.float32)


def kernel(x, edge_index, batch, W1, b1, W2, b2):
    """Full-input 2-layer GCN + mean-pool + log_softmax on 8 trn2 NeuronCores."""
    return np.asarray(
        run_gnn(np.asarray(x), np.asarray(edge_index), np.asarray(batch),
                np.asarray(W1), np.asarray(b1), np.asarray(W2), np.asarray(b2)),
        dtype=np.float32)


# revision 18
# speedup vs baseline: 1.0414x; 1.0414x over previous
"""GCN 2-layer + mean-pool + log_softmax kernel for 8x TRN2 cores.

v2 design (vs v1 baseline at 309us):
  - dinv[src] folded into xT on host (no on-chip y1 scaling)
  - replication via AllGather with flat-lowered APs into a dense row table
    (y1cc [12608 rows x 32B] per core, 64 embedded zero-pad rows), then one
    strided re-layout DMA into the 256B-stride gather table
  - gathers fetch int32 elements (8 x i32 = 32B for L1, 2 x i32 = 8B for L2)
  - scatter-add via fp8 DoubleRow matmuls (2 chunks / instruction)
  - per-block finalize relu(agg*dinv) on the Activation engine (scale AP)
  - stage C (h1 -> y2) per block: PE transpose + W2 matmul + DVE scale
  - host-precomputed one-hot S matrices (fixed + spill) uploaded as fp8
"""
import sys
for _p in ("/opt/trn_rl_repo", "/root/.axon_site/_ro/trn_rl_repo"):
    if _p not in sys.path:
        sys.path.append(_p)
import inspect
import numpy as np
import ml_dtypes

import concourse.bass as bass
import concourse.bacc as bacc
import concourse.mybir as mybir
import concourse.tile as tile

BF = ml_dtypes.bfloat16
F8 = ml_dtypes.float8_e4m3
P = 128
NCORES = 8
IN_CH = 256
HID = 32
OUT = 8
NG = 64
NGROUPS = 4
KSLOT = 4              # fixed slots per (lane, group)
LPC = P // KSLOT       # lanes per regular chunk
ZR = 64                # zero-pad rows embedded at the end of each core slab
SENT = 1000.0


def _install_patched_gather():
    if hasattr(bass.BassGpSimd, "dma_gather_p"):
        return True
    src = inspect.getsource(bass.BassGpSimd.dma_gather)
    src = src.replace(
        "elem_size_bytes > 0 and elem_size_bytes % 256 == 0",
        "elem_size_bytes > 0 and elem_size_bytes % 8 == 0")
    src = "def dma_gather_p" + src[src.index("("):]
    ns = dict(bass.__dict__)
    exec(compile(src, "dma_gather_p", "exec"), ns)
    bass.BassGpSimd.dma_gather_p = ns["dma_gather_p"]
    return True


def _collective_raw(eng, kind, op, in_ap, out_ap, replica_groups):
    """collective_compute with flat [[1,N],[1,1]] lowered APs (opt=False)."""
    from concourse.replica_groups import filter_and_check_groups
    eng.bass.has_collectives = True
    rg = filter_and_check_groups(eng.bass.num_devices, replica_groups)
    return eng.add_instruction(
        mybir.InstCollectiveCompute(
            name=f"I-{eng.bass.next_id()}",
            kind=kind, op=op, replica_groups=rg,
            ins=[eng.lower_ap(in_ap, opt=False)],
            outs=[eng.lower_ap(out_ap, opt=False)],
            unique_tensors="No", cc_dim="Partition"))


class Plan:
    """Uniform (core-independent) chunk schedule."""
    def __init__(self, bpc, nch_bg, call_plan, nsp_col_of, maxch):
        self.bpc = bpc
        self.npc = bpc * P
        self.slabr = self.npc + ZR
        self.winr = 2 * self.slabr
        self.trows = NCORES * self.slabr
        self.nch_bg = nch_bg            # [bpc][NGROUPS]
        self.call_plan = call_plan      # per group: list of (q_start, nchunks)
        self.nsp_col_of = nsp_col_of    # (b, g) -> first spill col
        self.nsp_cols = max(nsp_col_of.values(), default=-1) + 1 if nsp_col_of else 0
        self.maxch = maxch
        self.prefix_g = np.zeros((bpc + 1, NGROUPS), np.int64)
        for b in range(bpc):
            self.prefix_g[b + 1] = self.prefix_g[b] + nch_bg[b]
        self.call_col = {}
        col = 0
        for g in range(NGROUPS):
            lst = []
            for (q0, nch) in call_plan[g]:
                lst.append(col)
                col += nch * P // 16
            self.call_col[g] = lst
        self.idx_cols = col


def host_prep(x, edge_index, batch, maxch=96):
    N = x.shape[0]
    src = np.asarray(edge_index[0], np.int64)
    dst = np.asarray(edge_index[1], np.int64)
    batch = np.asarray(batch, np.int64)

    npc = -(-N // (NCORES * P)) * P
    bpc = npc // P
    slabr = npc + ZR
    winr = 2 * slabr
    assert winr <= 32768

    core = dst // npc
    blk = (dst % npc) // P
    lane = dst % P

    # block -> slot permutation balancing in-edge count across cores
    cnt_cb = np.bincount(core * bpc + blk, minlength=NCORES * bpc).reshape(NCORES, bpc)
    perm = np.argsort(-cnt_cb, axis=1, kind="stable")
    invperm = np.argsort(perm, axis=1)
    slot = invperm[core, blk]

    core_s = src // npc
    slot_src = invperm[core_s, (src % npc) // P]
    lane_src = src % P
    grp = core_s // 2
    rel = ((core_s % 2) * slabr + slot_src * P + lane_src).astype(np.int64)
    PAD = npc  # first zero row of the even slab, valid in every window

    # sort edges by (core, slot, group, lane)
    key = ((core * bpc + slot) * NGROUPS + grp) * P + lane
    order = np.argsort(key, kind="stable")
    key_s = key[order]
    rel_s = rel[order]
    nkeys = NCORES * bpc * NGROUPS * P
    cnt4 = np.bincount(key_s, minlength=nkeys)
    seg_start = np.concatenate([[0], np.cumsum(cnt4)])[:-1]
    ordinal = np.arange(len(key_s)) - seg_start[key_s]

    cnt4r = cnt4.reshape(NCORES, bpc, NGROUPS, P)
    spill = np.maximum(cnt4r - KSLOT, 0)
    spill_bg_c = spill.sum(axis=3)
    nspill_bg = -(-spill_bg_c.max(axis=0) // P)
    nch_bg = (KSLOT + nspill_bg).astype(np.int64)

    nsp_col_of = {}
    col = 0
    for b in range(bpc):
        for g in range(NGROUPS):
            if nspill_bg[b, g]:
                nsp_col_of[(b, g)] = col
                col += int(nspill_bg[b, g])
    nsp_cols = col

    chunks_g = (KSLOT + nspill_bg).sum(axis=0)
    call_plan = []
    for g in range(NGROUPS):
        ncg = int(chunks_g[g])
        calls, q = [], 0
        while q < ncg:
            n = min(maxch, ncg - q)
            calls.append((q, n))
            q += n
        call_plan.append(calls)

    plan = Plan(bpc, nch_bg.tolist(), call_plan, nsp_col_of, maxch)
    prefix_g = plan.prefix_g

    deg_full = np.bincount(dst, minlength=N).astype(np.float32) + 1.0
    dinv_full = 1.0 / np.sqrt(deg_full)

    core_e = key_s // (bpc * NGROUPS * P)
    rem = key_s % (bpc * NGROUPS * P)
    slot_e = rem // (NGROUPS * P)
    g_e = (rem // P) % NGROUPS
    lane_e = rem % P

    # fixed one-hot S (shared by all cores)
    s_ar = np.arange(P)
    sfx = np.zeros((P, KSLOT * P), F8)
    for r in range(KSLOT):
        sfx[s_ar, r * P + r * LPC + s_ar // KSLOT] = 1.0
    ident8 = np.eye(P, dtype=np.float32).astype(F8)
    identbf = np.eye(P, dtype=np.float32).astype(BF)

    per_core = []
    for c in range(NCORES):
        m = core_e == c
        sl, gg, ln, o, rr = slot_e[m], g_e[m], lane_e[m], ordinal[m], rel_s[m]

        idx_groups = [np.full((int(chunks_g[g]) * P,), PAD, np.int16)
                      for g in range(NGROUPS)]
        dl = np.full((max(nsp_cols, 1), P), -1, np.int64)

        regm = o < KSLOT
        q = prefix_g[sl[regm], gg[regm]] + ln[regm] // LPC
        pos = q * P + (ln[regm] % LPC) * KSLOT + o[regm]
        for g in range(NGROUPS):
            gm = gg[regm] == g
            idx_groups[g][pos[gm]] = rr[regm][gm].astype(np.int16)

        spm = ~regm
        if spm.any():
            key2 = sl[spm] * NGROUPS + gg[spm]
            cnt2 = np.bincount(key2, minlength=bpc * NGROUPS)
            st2 = np.concatenate([[0], np.cumsum(cnt2)])[:-1]
            so = np.arange(len(key2)) - st2[key2]
            qsp = prefix_g[sl[spm], gg[spm]] + KSLOT + so // P
            pos = qsp * P + so % P
            for g in range(NGROUPS):
                gm = gg[spm] == g
                idx_groups[g][pos[gm]] = rr[spm][gm].astype(np.int16)
            colbase = np.array([nsp_col_of.get((b, g), 0)
                                for b in range(bpc) for g in range(NGROUPS)],
                               np.int64).reshape(bpc, NGROUPS)
            cols = colbase[sl[spm], gg[spm]] + so // P
            dl[cols, so % P] = ln[spm]

        # spill one-hot S: [P rows, nsp_cols * P dst]
        ssp = np.zeros((P, max(nsp_cols, 1) * P), F8)
        qq, rrow = np.nonzero(dl >= 0)
        ssp[rrow, qq * P + dl[qq, rrow]] = 1.0

        cols_list = []
        for g in range(NGROUPS):
            arr = idx_groups[g]
            for (q0, nch) in call_plan[g]:
                seg = arr[q0 * P:(q0 + nch) * P]
                nid = nch * P
                w = np.zeros((16, nid // 16), np.int16)
                ii = np.arange(nid)
                w[ii % 16, ii // 16] = seg
                cols_list.append(np.tile(w, (8, 1)))
        idx_in = np.concatenate(cols_list, axis=1) if cols_list else np.zeros((P, 1), np.int16)

        # per-core node data (slot-permuted)
        nbase = c * npc
        dinvc = np.zeros((npc,), np.float32)
        hi = min(nbase + npc, N)
        if hi > nbase:
            dinvc[:hi - nbase] = dinv_full[nbase:hi]
        bl = np.full((npc,), -1, np.int64)
        if hi > nbase:
            bl[:hi - nbase] = batch[nbase:hi]
        dinvc = dinvc.reshape(bpc, P)[perm[c]].reshape(npc)
        bl = bl.reshape(bpc, P)[perm[c]].reshape(npc)
        dinv_t = dinvc.reshape(bpc, P).T.copy()               # [P, bpc]
        dinvrep8 = np.repeat(dinv_t[:, :, None], OUT, axis=2).reshape(P, bpc * OUT)
        # pooling one-hot: [P, bpc*NG]
        blm = bl.reshape(bpc, P).T                            # [P, bpc]
        sb = np.zeros((P, bpc, NG), np.float32)
        pi, bi = np.nonzero(blm >= 0)
        sb[pi, bi, blm[pi, bi]] = 1.0
        sb = sb.reshape(P, bpc * NG).astype(BF)

        per_core.append(dict(idx=idx_in, ssp=ssp, dinv=dinv_t,
                             dinv_l1=(dinv_t / 4.0).astype(np.float32),
                             dinvrep8=dinvrep8.astype(np.float32), sb=sb))

    cnts = np.bincount(batch[batch >= 0], minlength=NG).astype(np.float32)

    # xT with alpha*dinv folded, slot-permuted columns
    ALPHA = 4.0
    Np = npc * NCORES
    xs = np.asarray(x, np.float32) * (ALPHA * dinv_full)[:, None]
    xT = np.zeros((IN_CH, Np), F8)
    xT[:, :N] = xs.T.astype(F8)
    colperm = np.empty((Np,), np.int64)
    for c in range(NCORES):
        base = c * npc
        colperm[base:base + npc] = base + (perm[c][:, None] * P +
                                           np.arange(P)[None, :]).reshape(-1)
    xT = xT[:, colperm]
    return plan, per_core, cnts, xT, sfx, ident8, identbf


def build(plan: Plan, with_b1, with_b2):
    _install_patched_gather()
    nc = bacc.Bacc("TRN2", target_bir_lowering=False, debug=False,
                   num_swdge_queues=1, dynamic_dma_scratch_size=65536)
    f32, bf16, i16, i32 = (mybir.dt.float32, mybir.dt.bfloat16,
                           mybir.dt.int16, mybir.dt.int32)
    fp8 = mybir.dt.float8e4
    A = mybir.AluOpType
    AF = mybir.ActivationFunctionType
    DR = mybir.MatmulPerfMode.DoubleRow
    npc, bpc = plan.npc, plan.bpc
    SLABR, WINR, TROWS = plan.slabr, plan.winr, plan.trows
    NSP = max(plan.nsp_cols, 1)

    xT = nc.dram_tensor("xT", [IN_CH, npc], fp8, kind="ExternalInput")
    idx = nc.dram_tensor("idx", [P, plan.idx_cols], i16, kind="ExternalInput")
    sfx = nc.dram_tensor("sfx", [P, KSLOT * P], fp8, kind="ExternalInput")
    ssp = nc.dram_tensor("ssp", [P, NSP * P], fp8, kind="ExternalInput")
    ident8 = nc.dram_tensor("ident8", [P, P], fp8, kind="ExternalInput")
    identbf = nc.dram_tensor("identbf", [P, P], bf16, kind="ExternalInput")
    W1 = nc.dram_tensor("W1", [P, 2 * HID], f32, kind="ExternalInput")
    W2 = nc.dram_tensor("W2", [HID, OUT], f32, kind="ExternalInput")
    b1 = nc.dram_tensor("b1", [1, HID], f32, kind="ExternalInput")
    b2 = nc.dram_tensor("b2", [1, OUT], f32, kind="ExternalInput")
    dinv = nc.dram_tensor("dinv", [P, bpc], f32, kind="ExternalInput")
    dinv_l1 = nc.dram_tensor("dinv_l1", [P, bpc], f32, kind="ExternalInput")
    w2rep = nc.dram_tensor("w2rep", [P, OUT], f32, kind="ExternalInput")
    dinvrep8 = nc.dram_tensor("dinvrep8", [P, bpc * OUT], f32, kind="ExternalInput")
    sb = nc.dram_tensor("sb", [P, bpc * NG], bf16, kind="ExternalInput")
    out = nc.dram_tensor("out", [NG, OUT], f32, kind="ExternalOutput")
    import os as _os
    DBG = int(_os.environ.get("GNN_DEBUG", "0"))
    if DBG:
        y1dump = nc.dram_tensor("y1dump", [P, bpc * HID], f32, kind="ExternalOutput")
        h1dump = nc.dram_tensor("h1dump", [P, bpc * HID], f32, kind="ExternalOutput")
        y2dump = nc.dram_tensor("y2dump", [P, bpc * OUT], f32, kind="ExternalOutput")
        h2dump = nc.dram_tensor("h2dump", [P, bpc * OUT], f32, kind="ExternalOutput")
        windump = nc.dram_tensor("windump", [TROWS, HID], f32, kind="ExternalOutput")

    y1cc = nc.dram_tensor("y1cc", [SLABR, HID], fp8, kind="Internal")
    y1win = nc.dram_tensor("y1win", [TROWS, HID], fp8, kind="Internal")
    y1tab = nc.dram_tensor("y1tab", [TROWS, 256], fp8, kind="Internal")
    y2cc = nc.dram_tensor("y2cc", [SLABR, OUT], fp8, kind="Internal")
    y2win = nc.dram_tensor("y2win", [TROWS, OUT], fp8, kind="Internal")
    y2tab = nc.dram_tensor("y2tab", [TROWS, 256], fp8, kind="Internal")

    with tile.TileContext(nc) as tc:
        with tc.tile_pool(name="const", bufs=1) as cpool, \
             tc.tile_pool(name="persist", bufs=1) as pers, \
             tc.tile_pool(name="g0", bufs=2) as gp0, \
             tc.tile_pool(name="g1", bufs=2) as gp1, \
             tc.tile_pool(name="g2", bufs=2) as gp2, \
             tc.tile_pool(name="g3", bufs=2) as gp3, \
             tc.tile_pool(name="spool", bufs=8) as spool, \
             tc.tile_pool(name="psB", bufs=2, space="PSUM") as psB, \
             tc.tile_pool(name="psT", bufs=2, space="PSUM") as psT, \
             tc.tile_pool(name="ps2p", bufs=1, space="PSUM") as ps2p, \
             tc.tile_pool(name="psE", bufs=1, space="PSUM") as psE:
            gpools = [gp0, gp1, gp2, gp3]

            # ---- small constants on the scalar queue ----
            w1f = cpool.tile([P, 2 * HID], f32)
            nc.scalar.dma_start(out=w1f[:], in_=W1[:, :])
            w1t = cpool.tile([P, 2 * HID], bf16)
            nc.vector.tensor_copy(out=w1t[:], in_=w1f[:])
            w2f = cpool.tile([HID, OUT], f32)
            nc.scalar.dma_start(out=w2f[:], in_=W2[:, :])
            w2t = cpool.tile([HID, OUT], bf16)
            nc.vector.tensor_copy(out=w2t[:], in_=w2f[:])
            dinv_t = cpool.tile([P, bpc], f32)
            nc.scalar.dma_start(out=dinv_t[:], in_=dinv[:, :])
            dinvl1_t = cpool.tile([P, bpc], f32)
            nc.scalar.dma_start(out=dinvl1_t[:], in_=dinv_l1[:, :])
            w2rf = cpool.tile([P, OUT], f32)
            nc.scalar.dma_start(out=w2rf[:], in_=w2rep[:, :])
            w2rept = cpool.tile([P, OUT], bf16)
            nc.vector.tensor_copy(out=w2rept[:], in_=w2rf[:])
            dr8 = cpool.tile([P, bpc * OUT], f32)
            nc.scalar.dma_start(out=dr8[:], in_=dinvrep8[:, :])
            sfx_t = cpool.tile([P, KSLOT * P], fp8)
            nc.scalar.dma_start(out=sfx_t[:], in_=sfx[:, :])
            id8_t = cpool.tile([P, P], fp8)
            nc.scalar.dma_start(out=id8_t[:], in_=ident8[:, :])
            idbf_t = cpool.tile([P, P], bf16)
            nc.scalar.dma_start(out=idbf_t[:], in_=identbf[:, :])
            if with_b1 or with_b2:
                b1t = cpool.tile([1, HID], f32)
                nc.scalar.dma_start(out=b1t[:], in_=b1[:, :])
                b2t = cpool.tile([1, OUT], f32)
                nc.scalar.dma_start(out=b2t[:], in_=b2[:, :])
                ones_col = cpool.tile([1, P], f32)
                nc.gpsimd.memset(ones_col[:], 1.0)
                b1b_ps = psB.tile([P, HID], f32, name="b1b_ps")
                nc.tensor.matmul(out=b1b_ps[:], lhsT=ones_col[:], rhs=b1t[:],
                                 start=True, stop=True)
                b1b = cpool.tile([P, HID], f32)
                nc.vector.tensor_copy(out=b1b[:], in_=b1b_ps[:])
                b2b_ps = psB.tile([P, OUT], f32, name="b2b_ps")
                nc.tensor.matmul(out=b2b_ps[:], lhsT=ones_col[:], rhs=b2t[:],
                                 start=True, stop=True)
                b2b = cpool.tile([P, OUT], f32)
                nc.vector.tensor_copy(out=b2b[:], in_=b2b_ps[:])

            # idx + pooling one-hots (scalar queue, hidden under AG1)
            idx_t = pers.tile([P, plan.idx_cols], i16)
            nc.scalar.dma_start(out=idx_t[:], in_=idx[:, :])
            sb_t = pers.tile([P, bpc * NG], bf16)
            nc.scalar.dma_start(out=sb_t[:], in_=sb[:, :])

            ssp_t = pers.tile([P, NSP * P], fp8)

            # persistent activations
            y1_sh = pers.tile([P, bpc * HID], fp8)
            h1_sh = pers.tile([P, bpc * HID], bf16)
            y2_sh = pers.tile([P, bpc * OUT], fp8)
            h2_sh = pers.tile([P, bpc * OUT], bf16)

            # zero-pad rows of y1cc / y2cc (gpsimd queue)
            zf8 = cpool.tile([P, 64], fp8)
            nc.gpsimd.memset(zf8[:], 0.0)
            nc.gpsimd.dma_start(
                out=y1cc[npc:SLABR, :].rearrange("(a b) c -> a (b c)", a=ZR),
                in_=zf8[0:ZR, 0:HID])
            nc.gpsimd.dma_start(
                out=y2cc[npc:SLABR, :].rearrange("(a b) c -> a (b c)", a=ZR),
                in_=zf8[0:ZR, 0:OUT])

            # ---- stage A: y1 = (x*dinv) @ W1 ----
            SLAB = 8
            nslab = -(-bpc // SLAB)
            sbA_ctx = tc.tile_pool(name="sbA", bufs=2)
            sbA = sbA_ctx.__enter__()
            psA_ctx = tc.tile_pool(name="psA", bufs=2, space="PSUM")
            psA = psA_ctx.__enter__()

            def write_y1cc(piece):
                b0 = 0 if piece == 0 else 7 * SLAB
                b1_ = 7 * SLAB if piece == 0 else bpc
                nc.sync.dma_start(
                    out=y1cc[b0 * P:b1_ * P, :].rearrange("(b p) c -> p b c", p=P),
                    in_=y1_sh[:, b0 * HID:b1_ * HID].rearrange(
                        "p (b c) -> p b c", c=HID))

            for s in range(nslab):
                s0 = s * SLAB
                sbk = min(SLAB, bpc - s0)
                xt0 = sbA.tile([P, SLAB * P], fp8, tag="xt0")
                xt1 = sbA.tile([P, SLAB * P], fp8, tag="xt1")
                nc.sync.dma_start(out=xt0[:, :sbk * P], in_=xT[0:P, s0 * P:(s0 + sbk) * P])
                nc.gpsimd.dma_start(out=xt1[:, :sbk * P], in_=xT[P:2 * P, s0 * P:(s0 + sbk) * P])
                ps = psA.tile([P, SLAB * HID], f32, tag="pst")
                for j in range(sbk):
                    nc.tensor.matmul(out=ps[:, j * HID:(j + 1) * HID],
                                     lhsT=xt0[:, j * P:(j + 1) * P],
                                     rhs=w1t[:, 0:HID], start=True, stop=False)
                    nc.tensor.matmul(out=ps[:, j * HID:(j + 1) * HID],
                                     lhsT=xt1[:, j * P:(j + 1) * P],
                                     rhs=w1t[:, HID:2 * HID], start=False, stop=True)
                nc.vector.tensor_copy(out=y1_sh[:, s0 * HID:(s0 + sbk) * HID],
                                      in_=ps[:, :sbk * HID])
                if s == 6:
                    write_y1cc(0)
            write_y1cc(1)
            psA_ctx.__exit__(None, None, None)
            sbA_ctx.__exit__(None, None, None)

            # spill one-hots (sync queue, hidden under AG1)
            half = (NSP // 2) * P
            if half:
                nc.sync.dma_start(out=ssp_t[:, 0:half], in_=ssp[:, 0:half])
            nc.sync.dma_start(out=ssp_t[:, half:], in_=ssp[:, half:])

            # ---- replicate layer-1 table ----
            _collective_raw(nc.gpsimd, "AllGather", A.bypass,
                            y1cc[:, :], y1win[:, :], [list(range(NCORES))])
            TH = TROWS // 2
            nc.scalar.dma_start(out=y1tab[0:TH, 0:HID], in_=y1win[0:TH, :])
            nc.scalar.dma_start(out=y1tab[TH:TROWS, 0:HID], in_=y1win[TH:TROWS, :])

            # ---- aggregation ----
            def agg_layer(ytab, ysh, hsh, C, EL, gtag, with_b, bb, post_block,
                          dscale):
                gstate = [dict(call=-1, tile=None) for _ in range(NGROUPS)]

                def ensure_call(g, q):
                    cidx = 0
                    for i, (q0, nch) in enumerate(plan.call_plan[g]):
                        if q0 <= q < q0 + nch:
                            cidx = i
                            break
                    st = gstate[g]
                    if st["call"] == cidx:
                        return st["tile"], plan.call_plan[g][cidx][0], cidx
                    (q0, nch) = plan.call_plan[g][cidx]
                    gt = gpools[g].tile([P, plan.maxch * EL], i32,
                                        tag=f"{gtag}{g}", name=f"{gtag}t{g}")
                    col = plan.call_col[g][cidx]
                    nid = nch * P
                    nc.gpsimd.dma_gather_p(
                        out_ap=gt[:, :nch * EL].rearrange("p (k c) -> p k c", c=EL),
                        in_ap=ytab[g * WINR:(g + 1) * WINR, 0:EL * 4].bitcast(i32),
                        idxs_ap=idx_t[:, col:col + nid // 16],
                        num_idxs=nid, num_idxs_reg=nid,
                        elem_size=EL, elem_step=64, single_packet=False)
                    st["call"] = cidx
                    st["tile"] = gt
                    return gt, q0, cidx

                qcol = 0
                for b in range(bpc):
                    ps = psB.tile([P, C], f32, name="psB_t")
                    nchunks = sum(plan.nch_bg[b])
                    nc.tensor.matmul(out=ps[:], lhsT=id8_t[:],
                                     rhs=ysh[:, b * C:(b + 1) * C],
                                     start=True, stop=False)
                    done = 0
                    for g in range(NGROUPS):
                        nch_b = plan.nch_bg[b][g]
                        j = 0
                        while j < nch_b:
                            q = int(plan.prefix_g[b, g]) + j
                            gt, q0, cidx = ensure_call(g, q)
                            sl = q - q0
                            # pairable: next chunk exists, same call, same kind
                            import os as _os
                            same_kind = (j + 1 < nch_b and
                                         (j + 1 < KSLOT) == (j < KSLOT) and
                                         not int(_os.environ.get("GNN_NODR", "0")))
                            same_call = (q + 1 < plan.call_plan[g][cidx][0] +
                                         plan.call_plan[g][cidx][1])
                            if same_kind and same_call:
                                if j < KSLOT:
                                    S_ap = sfx_t[:, j * P:(j + 2) * P]
                                else:
                                    S_ap = ssp_t[:, qcol * P:(qcol + 2) * P]
                                    qcol += 2
                                done += 2
                                nc.tensor.matmul(
                                    out=ps[:],
                                    lhsT=S_ap.rearrange("p (two m) -> p two m", two=2),
                                    rhs=gt[:, sl * EL:(sl + 2) * EL].bitcast(fp8)
                                          .rearrange("p (two c) -> p two c", two=2),
                                    start=False, stop=(done == nchunks),
                                    perf_mode=DR)
                                j += 2
                            else:
                                if j < KSLOT:
                                    S_ap = sfx_t[:, j * P:(j + 1) * P]
                                else:
                                    S_ap = ssp_t[:, qcol * P:(qcol + 1) * P]
                                    qcol += 1
                                done += 1
                                nc.tensor.matmul(
                                    out=ps[:], lhsT=S_ap,
                                    rhs=gt[:, sl * EL:(sl + 1) * EL].bitcast(fp8)[:, 0:C],
                                    start=False, stop=(done == nchunks))
                                j += 1
                    dv = dscale[:, b:b + 1]
                    if with_b:
                        t1 = spool.tile([P, C], f32, name="t1_t")
                        nc.vector.tensor_scalar(
                            out=t1[:], in0=ps[:], scalar1=dv,
                            scalar2=None, op0=A.mult)
                        t2 = spool.tile([P, C], f32, name="t2_t")
                        nc.vector.tensor_tensor(out=t2[:], in0=t1[:],
                                                in1=bb[:, :C], op=A.add)
                        nc.scalar.activation(out=hsh[:, b * C:(b + 1) * C],
                                             in_=t2[:], func=AF.Relu)
                    elif b % 2 == 0:
                        nc.scalar.activation(out=hsh[:, b * C:(b + 1) * C],
                                             in_=ps[:], func=AF.Relu,
                                             scale=dv)
                    else:
                        nc.vector.tensor_scalar(
                            out=hsh[:, b * C:(b + 1) * C], in0=ps[:],
                            scalar1=dv, scalar2=0.0, op0=A.mult, op1=A.max)
                    if post_block is not None:
                        post_block(b)

            # stage C per block: transpose h1 block, @W2, scale -> y2_sh
            def stage_c(b):
                tp = psT.tile([HID, P], bf16, name="psT_t", tag="tp")
                nc.tensor.transpose(out=tp[:], in_=h1_sh[:, b * HID:(b + 1) * HID],
                                    identity=idbf_t[:])
                h1Tb = spool.tile([HID, P], bf16, name="h1Tb_t")
                nc.vector.tensor_copy(out=h1Tb[:], in_=tp[:])
                ps2 = ps2p.tile([P, OUT], f32, name="ps2_t")
                nc.tensor.matmul(out=ps2[:], lhsT=h1Tb[:],
                                 rhs=w2t[:], start=True, stop=True)
                nc.vector.tensor_tensor(
                    out=y2_sh[:, b * OUT:(b + 1) * OUT], in0=ps2[:],
                    in1=dr8[:, b * OUT:(b + 1) * OUT], op=A.mult)
                if b == 7 * SLAB - 1:
                    write_y2cc(0)

            def write_y2cc(piece):
                b0 = 0 if piece == 0 else 7 * SLAB
                b1_ = 7 * SLAB if piece == 0 else bpc
                nc.sync.dma_start(
                    out=y2cc[b0 * P:b1_ * P, :].rearrange("(b p) c -> p b c", p=P),
                    in_=y2_sh[:, b0 * OUT:b1_ * OUT].rearrange(
                        "p (b c) -> p b c", c=OUT))

            agg_layer(y1tab, y1_sh, h1_sh, HID, 8, "ga", with_b1,
                      b1b if with_b1 else None, stage_c, dinvl1_t)
            write_y2cc(1)

            # ---- replicate layer-2 table ----
            _collective_raw(nc.gpsimd, "AllGather", A.bypass,
                            y2cc[:, :], y2win[:, :], [list(range(NCORES))])
            nc.scalar.dma_start(out=y2tab[0:TH, 0:OUT], in_=y2win[0:TH, :])
            nc.scalar.dma_start(out=y2tab[TH:TROWS, 0:OUT], in_=y2win[TH:TROWS, :])

            # layer 2 + pooling per block
            pp = psE.tile([NG, OUT], f32)

            def pool_block(b):
                nc.tensor.matmul(out=pp[:], lhsT=sb_t[:, b * NG:(b + 1) * NG],
                                 rhs=h2_sh[:, b * OUT:(b + 1) * OUT],
                                 start=(b == 0), stop=(b == bpc - 1))

            agg_layer(y2tab, y2_sh, h2_sh, OUT, 2, "gb", with_b2,
                      b2b if with_b2 else None, pool_block, dinv_t)

            sums = cpool.tile([NG, OUT], f32)
            nc.vector.tensor_copy(out=sums[:], in_=pp[:])
            nc.sync.dma_start(out=out[:, :], in_=sums[:])
            if DBG:
                TH2 = TROWS // 2
                nc.gpsimd.dma_start(out=windump[0:TH2, :], in_=y1win[0:TH2, :])
                nc.gpsimd.dma_start(out=windump[TH2:TROWS, :], in_=y1win[TH2:TROWS, :])
                nc.gpsimd.dma_start(out=y1dump[:, :], in_=y1_sh[:])
                nc.gpsimd.dma_start(out=h1dump[:, :], in_=h1_sh[:])
                nc.gpsimd.dma_start(out=y2dump[:, :], in_=y2_sh[:])
                nc.gpsimd.dma_start(out=h2dump[:, :], in_=h2_sh[:])

    nc.compile()
    return nc


def prep_program(x, edge_index, batch, W1, b1, W2, b2):
    """Build the compiled program + per-core input maps + pooling counts."""
    plan, per_core, cnts, xT, sfx, ident8, identbf = host_prep(x, edge_index, batch)
    with_b1 = bool(np.any(np.asarray(b1)))
    with_b2 = bool(np.any(np.asarray(b2)))
    nc = build(plan, with_b1, with_b2)
    W1a = np.asarray(W1, np.float32)
    w1h = np.concatenate([W1a[0:P, :], W1a[P:2 * P, :]], axis=1)  # [P, 2*HID]
    W2a = np.asarray(W2, np.float32)
    w2r = np.zeros((P, OUT), np.float32)
    for j in range(4):
        w2r[j * HID:(j + 1) * HID] = W2a
    in_maps = []
    for c in range(NCORES):
        pc = per_core[c]
        m = dict(
            xT=np.ascontiguousarray(xT[:, c * plan.npc:(c + 1) * plan.npc]),
            idx=pc["idx"], sfx=sfx, ssp=pc["ssp"], ident8=ident8,
            identbf=identbf, W1=w1h, W2=np.asarray(W2, np.float32),
            b1=np.asarray(b1, np.float32).reshape(1, HID),
            b2=np.asarray(b2, np.float32).reshape(1, OUT),
            dinv=pc["dinv"], dinv_l1=pc["dinv_l1"],
            dinvrep8=pc["dinvrep8"], sb=pc["sb"], w2rep=w2r)
        in_maps.append(m)
    return nc, in_maps, cnts


def run_gnn(x, edge_index, batch, W1, b1, W2, b2):
    from concourse.bass_utils import run_bass_kernel_spmd
    nc, in_maps, cnts = prep_program(x, edge_index, batch, W1, b1, W2, b2)
    res = run_bass_kernel_spmd(nc, in_maps, core_ids=list(range(NCORES)))
    total = np.zeros((NG, OUT), np.float64)
    for c in range(NCORES):
        total += np.asarray(res.results[c]["out"], np.float64)
    pooled = total / np.maximum(cnts, 1.0)[:, None]
    z = pooled - pooled.max(axis=1, keepdims=True)
    ls = z - np.log(np.exp(z).sum(axis=1, keepdims=True))
    return ls.astype(np.float32)


def kernel(x, edge_index, batch, W1, b1, W2, b2):
    """Full-input 2-layer GCN + mean-pool + log_softmax on 8 trn2 NeuronCores."""
    return np.asarray(
        run_gnn(np.asarray(x), np.asarray(edge_index), np.asarray(batch),
                np.asarray(W1), np.asarray(b1), np.asarray(W2), np.asarray(b2)),
        dtype=np.float32)


# revision 20
# speedup vs baseline: 1.0810x; 1.0380x over previous
"""GCN 2-layer + mean-pool + log_softmax kernel for 8x TRN2 cores.

v2 design (vs v1 baseline at 309us):
  - dinv[src] folded into xT on host (no on-chip y1 scaling)
  - replication via AllGather with flat-lowered APs into a dense row table
    (y1cc [12608 rows x 32B] per core, 64 embedded zero-pad rows), then one
    strided re-layout DMA into the 256B-stride gather table
  - gathers fetch int32 elements (8 x i32 = 32B for L1, 2 x i32 = 8B for L2)
  - scatter-add via fp8 DoubleRow matmuls (2 chunks / instruction)
  - per-block finalize relu(agg*dinv) on the Activation engine (scale AP)
  - stage C (h1 -> y2) per block: PE transpose + W2 matmul + DVE scale
  - host-precomputed one-hot S matrices (fixed + spill) uploaded as fp8
"""
import sys
for _p in ("/opt/trn_rl_repo", "/root/.axon_site/_ro/trn_rl_repo"):
    if _p not in sys.path:
        sys.path.append(_p)
import inspect
import numpy as np
import ml_dtypes

import concourse.bass as bass
import concourse.bacc as bacc
import concourse.mybir as mybir
import concourse.tile as tile

BF = ml_dtypes.bfloat16
F8 = ml_dtypes.float8_e4m3
P = 128
NCORES = 8
IN_CH = 256
HID = 32
OUT = 8
NG = 64
NGROUPS = 4
KSLOT = 4              # fixed slots per (lane, group)
LPC = P // KSLOT       # lanes per regular chunk
ZR = 64                # zero-pad rows embedded at the end of each core slab
SENT = 1000.0


def _install_patched_gather():
    if hasattr(bass.BassGpSimd, "dma_gather_p"):
        return True
    src = inspect.getsource(bass.BassGpSimd.dma_gather)
    src = src.replace(
        "elem_size_bytes > 0 and elem_size_bytes % 256 == 0",
        "elem_size_bytes > 0 and elem_size_bytes % 8 == 0")
    src = "def dma_gather_p" + src[src.index("("):]
    ns = dict(bass.__dict__)
    exec(compile(src, "dma_gather_p", "exec"), ns)
    bass.BassGpSimd.dma_gather_p = ns["dma_gather_p"]
    return True


def _collective_raw(eng, kind, op, in_ap, out_ap, replica_groups):
    """collective_compute with flat [[1,N],[1,1]] lowered APs (opt=False)."""
    from concourse.replica_groups import filter_and_check_groups
    eng.bass.has_collectives = True
    rg = filter_and_check_groups(eng.bass.num_devices, replica_groups)
    return eng.add_instruction(
        mybir.InstCollectiveCompute(
            name=f"I-{eng.bass.next_id()}",
            kind=kind, op=op, replica_groups=rg,
            ins=[eng.lower_ap(in_ap, opt=False)],
            outs=[eng.lower_ap(out_ap, opt=False)],
            unique_tensors="No", cc_dim="Partition"))


class Plan:
    """Uniform (core-independent) chunk schedule."""
    def __init__(self, bpc, nch_bg, call_plan, nsp_col_of, maxch):
        self.bpc = bpc
        self.npc = bpc * P
        self.slabr = self.npc + ZR
        self.winr = 2 * self.slabr
        self.trows = NCORES * self.slabr
        self.nch_bg = nch_bg            # [bpc][NGROUPS]
        self.call_plan = call_plan      # per group: list of (q_start, nchunks)
        self.nsp_col_of = nsp_col_of    # (b, g) -> first spill col
        self.nsp_cols = max(nsp_col_of.values(), default=-1) + 1 if nsp_col_of else 0
        self.maxch = maxch
        self.prefix_g = np.zeros((bpc + 1, NGROUPS), np.int64)
        for b in range(bpc):
            self.prefix_g[b + 1] = self.prefix_g[b] + nch_bg[b]
        self.call_col = {}
        col = 0
        for g in range(NGROUPS):
            lst = []
            for (q0, nch) in call_plan[g]:
                lst.append(col)
                col += nch * P // 16
            self.call_col[g] = lst
        self.idx_cols = col


def host_prep(x, edge_index, batch, maxch=96):
    N = x.shape[0]
    src = np.asarray(edge_index[0], np.int64)
    dst = np.asarray(edge_index[1], np.int64)
    batch = np.asarray(batch, np.int64)

    npc = -(-N // (NCORES * P)) * P
    bpc = npc // P
    slabr = npc + ZR
    winr = 2 * slabr
    assert winr <= 32768

    core = dst // npc
    blk = (dst % npc) // P
    lane = dst % P

    # block -> slot permutation balancing in-edge count across cores
    cnt_cb = np.bincount(core * bpc + blk, minlength=NCORES * bpc).reshape(NCORES, bpc)
    perm = np.argsort(-cnt_cb, axis=1, kind="stable")
    invperm = np.argsort(perm, axis=1)
    slot = invperm[core, blk]

    core_s = src // npc
    slot_src = invperm[core_s, (src % npc) // P]
    lane_src = src % P
    grp = core_s // 2
    rel = ((core_s % 2) * slabr + slot_src * P + lane_src).astype(np.int64)
    PAD = npc  # first zero row of the even slab, valid in every window

    # sort edges by (core, slot, group, lane)
    key = ((core * bpc + slot) * NGROUPS + grp) * P + lane
    order = np.argsort(key, kind="stable")
    key_s = key[order]
    rel_s = rel[order]
    nkeys = NCORES * bpc * NGROUPS * P
    cnt4 = np.bincount(key_s, minlength=nkeys)
    seg_start = np.concatenate([[0], np.cumsum(cnt4)])[:-1]
    ordinal = np.arange(len(key_s)) - seg_start[key_s]

    cnt4r = cnt4.reshape(NCORES, bpc, NGROUPS, P)
    spill = np.maximum(cnt4r - KSLOT, 0)
    spill_bg_c = spill.sum(axis=3)
    nspill_bg = -(-spill_bg_c.max(axis=0) // P)
    nch_bg = (KSLOT + nspill_bg).astype(np.int64)

    nsp_col_of = {}
    col = 0
    for b in range(bpc):
        for g in range(NGROUPS):
            if nspill_bg[b, g]:
                nsp_col_of[(b, g)] = col
                col += int(nspill_bg[b, g])
    nsp_cols = col

    chunks_g = (KSLOT + nspill_bg).sum(axis=0)
    call_plan = []
    for g in range(NGROUPS):
        ncg = int(chunks_g[g])
        calls, q = [], 0
        while q < ncg:
            n = min(maxch, ncg - q)
            calls.append((q, n))
            q += n
        call_plan.append(calls)

    plan = Plan(bpc, nch_bg.tolist(), call_plan, nsp_col_of, maxch)
    prefix_g = plan.prefix_g

    deg_full = np.bincount(dst, minlength=N).astype(np.float32) + 1.0
    dinv_full = 1.0 / np.sqrt(deg_full)

    core_e = key_s // (bpc * NGROUPS * P)
    rem = key_s % (bpc * NGROUPS * P)
    slot_e = rem // (NGROUPS * P)
    g_e = (rem // P) % NGROUPS
    lane_e = rem % P

    # fixed one-hot S (shared by all cores)
    s_ar = np.arange(P)
    sfx = np.zeros((P, KSLOT * P), F8)
    for r in range(KSLOT):
        sfx[s_ar, r * P + r * LPC + s_ar // KSLOT] = 1.0
    ident8 = np.eye(P, dtype=np.float32).astype(F8)
    identbf = np.eye(P, dtype=np.float32).astype(BF)

    per_core = []
    for c in range(NCORES):
        m = core_e == c
        sl, gg, ln, o, rr = slot_e[m], g_e[m], lane_e[m], ordinal[m], rel_s[m]

        idx_groups = [np.full((int(chunks_g[g]) * P,), PAD, np.int16)
                      for g in range(NGROUPS)]
        dl = np.full((max(nsp_cols, 1), P), -1, np.int64)

        regm = o < KSLOT
        q = prefix_g[sl[regm], gg[regm]] + ln[regm] // LPC
        pos = q * P + (ln[regm] % LPC) * KSLOT + o[regm]
        for g in range(NGROUPS):
            gm = gg[regm] == g
            idx_groups[g][pos[gm]] = rr[regm][gm].astype(np.int16)

        spm = ~regm
        if spm.any():
            key2 = sl[spm] * NGROUPS + gg[spm]
            cnt2 = np.bincount(key2, minlength=bpc * NGROUPS)
            st2 = np.concatenate([[0], np.cumsum(cnt2)])[:-1]
            so = np.arange(len(key2)) - st2[key2]
            qsp = prefix_g[sl[spm], gg[spm]] + KSLOT + so // P
            pos = qsp * P + so % P
            for g in range(NGROUPS):
                gm = gg[spm] == g
                idx_groups[g][pos[gm]] = rr[spm][gm].astype(np.int16)
            colbase = np.array([nsp_col_of.get((b, g), 0)
                                for b in range(bpc) for g in range(NGROUPS)],
                               np.int64).reshape(bpc, NGROUPS)
            cols = colbase[sl[spm], gg[spm]] + so // P
            dl[cols, so % P] = ln[spm]

        # spill one-hot S: [P rows, nsp_cols * P dst]
        ssp = np.zeros((P, max(nsp_cols, 1) * P), F8)
        qq, rrow = np.nonzero(dl >= 0)
        ssp[rrow, qq * P + dl[qq, rrow]] = 1.0

        cols_list = []
        for g in range(NGROUPS):
            arr = idx_groups[g]
            for (q0, nch) in call_plan[g]:
                seg = arr[q0 * P:(q0 + nch) * P]
                nid = nch * P
                w = np.zeros((16, nid // 16), np.int16)
                ii = np.arange(nid)
                w[ii % 16, ii // 16] = seg
                cols_list.append(np.tile(w, (8, 1)))
        idx_in = np.concatenate(cols_list, axis=1) if cols_list else np.zeros((P, 1), np.int16)

        # per-core node data (slot-permuted)
        nbase = c * npc
        dinvc = np.zeros((npc,), np.float32)
        hi = min(nbase + npc, N)
        if hi > nbase:
            dinvc[:hi - nbase] = dinv_full[nbase:hi]
        bl = np.full((npc,), -1, np.int64)
        if hi > nbase:
            bl[:hi - nbase] = batch[nbase:hi]
        dinvc = dinvc.reshape(bpc, P)[perm[c]].reshape(npc)
        bl = bl.reshape(bpc, P)[perm[c]].reshape(npc)
        dinv_t = dinvc.reshape(bpc, P).T.copy()               # [P, bpc]
        dinvrep8 = np.repeat(dinv_t[:, :, None], OUT, axis=2).reshape(P, bpc * OUT)
        # pooling one-hot: [P, bpc*NG]
        blm = bl.reshape(bpc, P).T                            # [P, bpc]
        sb = np.zeros((P, bpc, NG), np.float32)
        pi, bi = np.nonzero(blm >= 0)
        sb[pi, bi, blm[pi, bi]] = 1.0
        sb = sb.reshape(P, bpc * NG).astype(BF)

        per_core.append(dict(idx=idx_in, ssp=ssp, dinv=dinv_t,
                             dinv_l1=(dinv_t * dinv_t / 4.0).astype(np.float32),
                             dinvrep8=dinvrep8.astype(np.float32), sb=sb))

    cnts = np.bincount(batch[batch >= 0], minlength=NG).astype(np.float32)

    # xT with alpha*dinv folded, slot-permuted columns
    ALPHA = 4.0
    Np = npc * NCORES
    xs = np.asarray(x, np.float32) * (ALPHA * dinv_full)[:, None]
    xT = np.zeros((IN_CH, Np), F8)
    xT[:, :N] = xs.T.astype(F8)
    colperm = np.empty((Np,), np.int64)
    for c in range(NCORES):
        base = c * npc
        colperm[base:base + npc] = base + (perm[c][:, None] * P +
                                           np.arange(P)[None, :]).reshape(-1)
    xT = xT[:, colperm]
    return plan, per_core, cnts, xT, sfx, ident8, identbf


def build(plan: Plan, with_b1, with_b2):
    _install_patched_gather()
    nc = bacc.Bacc("TRN2", target_bir_lowering=False, debug=False,
                   num_swdge_queues=1, dynamic_dma_scratch_size=65536)
    f32, bf16, i16, i32 = (mybir.dt.float32, mybir.dt.bfloat16,
                           mybir.dt.int16, mybir.dt.int32)
    fp8 = mybir.dt.float8e4
    A = mybir.AluOpType
    AF = mybir.ActivationFunctionType
    DR = mybir.MatmulPerfMode.DoubleRow
    npc, bpc = plan.npc, plan.bpc
    SLABR, WINR, TROWS = plan.slabr, plan.winr, plan.trows
    NSP = max(plan.nsp_cols, 1)

    xT = nc.dram_tensor("xT", [IN_CH, npc], fp8, kind="ExternalInput")
    idx = nc.dram_tensor("idx", [P, plan.idx_cols], i16, kind="ExternalInput")
    sfx = nc.dram_tensor("sfx", [P, KSLOT * P], fp8, kind="ExternalInput")
    ssp = nc.dram_tensor("ssp", [P, NSP * P], fp8, kind="ExternalInput")
    ident8 = nc.dram_tensor("ident8", [P, P], fp8, kind="ExternalInput")
    identbf = nc.dram_tensor("identbf", [P, P], bf16, kind="ExternalInput")
    W1 = nc.dram_tensor("W1", [P, 2 * HID], f32, kind="ExternalInput")
    W2 = nc.dram_tensor("W2", [HID, OUT], f32, kind="ExternalInput")
    b1 = nc.dram_tensor("b1", [1, HID], f32, kind="ExternalInput")
    b2 = nc.dram_tensor("b2", [1, OUT], f32, kind="ExternalInput")
    dinv = nc.dram_tensor("dinv", [P, bpc], f32, kind="ExternalInput")
    dinv_l1 = nc.dram_tensor("dinv_l1", [P, bpc], f32, kind="ExternalInput")
    w2rep = nc.dram_tensor("w2rep", [P, OUT], f32, kind="ExternalInput")
    dinvrep8 = nc.dram_tensor("dinvrep8", [P, bpc * OUT], f32, kind="ExternalInput")
    sb = nc.dram_tensor("sb", [P, bpc * NG], bf16, kind="ExternalInput")
    out = nc.dram_tensor("out", [NG, OUT], f32, kind="ExternalOutput")
    import os as _os
    DBG = int(_os.environ.get("GNN_DEBUG", "0"))
    if DBG:
        y1dump = nc.dram_tensor("y1dump", [P, bpc * HID], f32, kind="ExternalOutput")
        h1dump = nc.dram_tensor("h1dump", [P, bpc * HID], f32, kind="ExternalOutput")
        y2dump = nc.dram_tensor("y2dump", [P, bpc * OUT], f32, kind="ExternalOutput")
        h2dump = nc.dram_tensor("h2dump", [P, bpc * OUT], f32, kind="ExternalOutput")
        windump = nc.dram_tensor("windump", [TROWS, HID], f32, kind="ExternalOutput")

    y1cc = nc.dram_tensor("y1cc", [SLABR, HID], fp8, kind="Internal")
    y1win = nc.dram_tensor("y1win", [TROWS, HID], fp8, kind="Internal")
    y1tab = nc.dram_tensor("y1tab", [TROWS, 256], fp8, kind="Internal")
    y2cc = nc.dram_tensor("y2cc", [SLABR, OUT], fp8, kind="Internal")
    y2win = nc.dram_tensor("y2win", [TROWS, OUT], fp8, kind="Internal")
    y2tab = nc.dram_tensor("y2tab", [TROWS, 256], fp8, kind="Internal")

    with tile.TileContext(nc) as tc:
        with tc.tile_pool(name="const", bufs=1) as cpool, \
             tc.tile_pool(name="persist", bufs=1) as pers, \
             tc.tile_pool(name="g0", bufs=2) as gp0, \
             tc.tile_pool(name="g1", bufs=2) as gp1, \
             tc.tile_pool(name="g2", bufs=2) as gp2, \
             tc.tile_pool(name="g3", bufs=2) as gp3, \
             tc.tile_pool(name="spool", bufs=8) as spool, \
             tc.tile_pool(name="psB", bufs=2, space="PSUM") as psB, \
             tc.tile_pool(name="psT", bufs=2, space="PSUM") as psT, \
             tc.tile_pool(name="ps2p", bufs=1, space="PSUM") as ps2p, \
             tc.tile_pool(name="psE", bufs=1, space="PSUM") as psE:
            gpools = [gp0, gp1, gp2, gp3]

            # ---- small constants on the scalar queue ----
            w1f = cpool.tile([P, 2 * HID], f32)
            nc.scalar.dma_start(out=w1f[:], in_=W1[:, :])
            w1t = cpool.tile([P, 2 * HID], bf16)
            nc.vector.tensor_copy(out=w1t[:], in_=w1f[:])
            w2f = cpool.tile([HID, OUT], f32)
            nc.scalar.dma_start(out=w2f[:], in_=W2[:, :])
            w2t = cpool.tile([HID, OUT], bf16)
            nc.vector.tensor_copy(out=w2t[:], in_=w2f[:])
            dinv_t = cpool.tile([P, bpc], f32)
            nc.scalar.dma_start(out=dinv_t[:], in_=dinv[:, :])
            dinvl1_t = cpool.tile([P, bpc], f32)
            nc.scalar.dma_start(out=dinvl1_t[:], in_=dinv_l1[:, :])
            w2rf = cpool.tile([P, OUT], f32)
            nc.scalar.dma_start(out=w2rf[:], in_=w2rep[:, :])
            w2rept = cpool.tile([P, OUT], bf16)
            nc.vector.tensor_copy(out=w2rept[:], in_=w2rf[:])
            dr8 = cpool.tile([P, bpc * OUT], f32)
            nc.scalar.dma_start(out=dr8[:], in_=dinvrep8[:, :])
            sfx_t = cpool.tile([P, KSLOT * P], fp8)
            nc.scalar.dma_start(out=sfx_t[:], in_=sfx[:, :])
            id8_t = cpool.tile([P, P], fp8)
            nc.scalar.dma_start(out=id8_t[:], in_=ident8[:, :])
            idbf_t = cpool.tile([P, P], bf16)
            nc.scalar.dma_start(out=idbf_t[:], in_=identbf[:, :])
            if with_b1 or with_b2:
                b1t = cpool.tile([1, HID], f32)
                nc.scalar.dma_start(out=b1t[:], in_=b1[:, :])
                b2t = cpool.tile([1, OUT], f32)
                nc.scalar.dma_start(out=b2t[:], in_=b2[:, :])
                ones_col = cpool.tile([1, P], f32)
                nc.gpsimd.memset(ones_col[:], 1.0)
                b1b_ps = psB.tile([P, HID], f32, name="b1b_ps")
                nc.tensor.matmul(out=b1b_ps[:], lhsT=ones_col[:], rhs=b1t[:],
                                 start=True, stop=True)
                b1b = cpool.tile([P, HID], f32)
                nc.vector.tensor_copy(out=b1b[:], in_=b1b_ps[:])
                b2b_ps = psB.tile([P, OUT], f32, name="b2b_ps")
                nc.tensor.matmul(out=b2b_ps[:], lhsT=ones_col[:], rhs=b2t[:],
                                 start=True, stop=True)
                b2b = cpool.tile([P, OUT], f32)
                nc.vector.tensor_copy(out=b2b[:], in_=b2b_ps[:])

            # idx + pooling one-hots (scalar queue, hidden under AG1)
            idx_t = pers.tile([P, plan.idx_cols], i16)
            nc.scalar.dma_start(out=idx_t[:], in_=idx[:, :])
            sb_t = pers.tile([P, bpc * NG], bf16)
            nc.scalar.dma_start(out=sb_t[:], in_=sb[:, :])

            ssp_t = pers.tile([P, NSP * P], fp8)

            # persistent activations
            y1_sh = pers.tile([P, bpc * HID], fp8)
            h1_sh = pers.tile([P, bpc * HID], bf16)
            y2_sh = pers.tile([P, bpc * OUT], fp8)
            h2_sh = pers.tile([P, bpc * OUT], bf16)

            # zero-pad rows of y1cc / y2cc (gpsimd queue)
            zf8 = cpool.tile([P, 64], fp8)
            nc.gpsimd.memset(zf8[:], 0.0)
            nc.gpsimd.dma_start(
                out=y1cc[npc:SLABR, :].rearrange("(a b) c -> a (b c)", a=ZR),
                in_=zf8[0:ZR, 0:HID])
            nc.gpsimd.dma_start(
                out=y2cc[npc:SLABR, :].rearrange("(a b) c -> a (b c)", a=ZR),
                in_=zf8[0:ZR, 0:OUT])

            # ---- stage A: y1 = (x*dinv) @ W1 ----
            SLAB = 8
            nslab = -(-bpc // SLAB)
            sbA_ctx = tc.tile_pool(name="sbA", bufs=2)
            sbA = sbA_ctx.__enter__()
            psA_ctx = tc.tile_pool(name="psA", bufs=2, space="PSUM")
            psA = psA_ctx.__enter__()

            def write_y1cc(piece):
                b0 = 0 if piece == 0 else 7 * SLAB
                b1_ = 7 * SLAB if piece == 0 else bpc
                nc.scalar.dma_start(
                    out=y1cc[b0 * P:b1_ * P, :].rearrange("(b p) c -> p b c", p=P),
                    in_=y1_sh[:, b0 * HID:b1_ * HID].rearrange(
                        "p (b c) -> p b c", c=HID))

            for s in range(nslab):
                s0 = s * SLAB
                sbk = min(SLAB, bpc - s0)
                xt0 = sbA.tile([P, SLAB * P], fp8, tag="xt0")
                xt1 = sbA.tile([P, SLAB * P], fp8, tag="xt1")
                nc.gpsimd.dma_start(out=xt0[:, :sbk * P], in_=xT[0:P, s0 * P:(s0 + sbk) * P])
                nc.gpsimd.dma_start(out=xt1[:, :sbk * P], in_=xT[P:2 * P, s0 * P:(s0 + sbk) * P])
                ps = psA.tile([P, SLAB * HID], f32, tag="pst")
                for j in range(sbk):
                    nc.tensor.matmul(out=ps[:, j * HID:(j + 1) * HID],
                                     lhsT=xt0[:, j * P:(j + 1) * P],
                                     rhs=w1t[:, 0:HID], start=True, stop=False)
                    nc.tensor.matmul(out=ps[:, j * HID:(j + 1) * HID],
                                     lhsT=xt1[:, j * P:(j + 1) * P],
                                     rhs=w1t[:, HID:2 * HID], start=False, stop=True)
                nc.vector.tensor_copy(out=y1_sh[:, s0 * HID:(s0 + sbk) * HID],
                                      in_=ps[:, :sbk * HID])
                if s == 6:
                    write_y1cc(0)
            write_y1cc(1)
            psA_ctx.__exit__(None, None, None)
            sbA_ctx.__exit__(None, None, None)

            # spill one-hots (sync queue, hidden under AG1)
            half = (NSP // 2) * P
            if half:
                nc.sync.dma_start(out=ssp_t[:, 0:half], in_=ssp[:, 0:half])
            nc.sync.dma_start(out=ssp_t[:, half:], in_=ssp[:, half:])

            # ---- replicate layer-1 table ----
            _collective_raw(nc.gpsimd, "AllGather", A.bypass,
                            y1cc[:, :], y1win[:, :], [list(range(NCORES))])
            TH = TROWS // 2
            nc.scalar.dma_start(out=y1tab[0:TH, 0:HID], in_=y1win[0:TH, :])
            nc.scalar.dma_start(out=y1tab[TH:TROWS, 0:HID], in_=y1win[TH:TROWS, :])

            # ---- aggregation ----
            def agg_layer(ytab, ysh, hsh, C, EL, gtag, with_b, bb, post_block,
                          dscale):
                gstate = [dict(call=-1, tile=None) for _ in range(NGROUPS)]

                def ensure_call(g, q):
                    cidx = 0
                    for i, (q0, nch) in enumerate(plan.call_plan[g]):
                        if q0 <= q < q0 + nch:
                            cidx = i
                            break
                    st = gstate[g]
                    if st["call"] == cidx:
                        return st["tile"], plan.call_plan[g][cidx][0], cidx
                    (q0, nch) = plan.call_plan[g][cidx]
                    gt = gpools[g].tile([P, plan.maxch * EL], i32,
                                        tag=f"{gtag}{g}", name=f"{gtag}t{g}")
                    col = plan.call_col[g][cidx]
                    nid = nch * P
                    nc.gpsimd.dma_gather_p(
                        out_ap=gt[:, :nch * EL].rearrange("p (k c) -> p k c", c=EL),
                        in_ap=ytab[g * WINR:(g + 1) * WINR, 0:EL * 4].bitcast(i32),
                        idxs_ap=idx_t[:, col:col + nid // 16],
                        num_idxs=nid, num_idxs_reg=nid,
                        elem_size=EL, elem_step=64, single_packet=False)
                    st["call"] = cidx
                    st["tile"] = gt
                    return gt, q0, cidx

                qcol = 0
                for b in range(bpc):
                    ps = psB.tile([P, C], f32, name="psB_t")
                    nchunks = sum(plan.nch_bg[b])
                    nc.tensor.matmul(out=ps[:], lhsT=id8_t[:],
                                     rhs=ysh[:, b * C:(b + 1) * C],
                                     start=True, stop=False)
                    done = 0
                    for g in range(NGROUPS):
                        nch_b = plan.nch_bg[b][g]
                        j = 0
                        while j < nch_b:
                            q = int(plan.prefix_g[b, g]) + j
                            gt, q0, cidx = ensure_call(g, q)
                            sl = q - q0
                            # pairable: next chunk exists, same call, same kind
                            import os as _os
                            same_kind = (j + 1 < nch_b and
                                         (j + 1 < KSLOT) == (j < KSLOT) and
                                         not int(_os.environ.get("GNN_NODR", "0")))
                            same_call = (q + 1 < plan.call_plan[g][cidx][0] +
                                         plan.call_plan[g][cidx][1])
                            if same_kind and same_call:
                                if j < KSLOT:
                                    S_ap = sfx_t[:, j * P:(j + 2) * P]
                                else:
                                    S_ap = ssp_t[:, qcol * P:(qcol + 2) * P]
                                    qcol += 2
                                done += 2
                                nc.tensor.matmul(
                                    out=ps[:],
                                    lhsT=S_ap.rearrange("p (two m) -> p two m", two=2),
                                    rhs=gt[:, sl * EL:(sl + 2) * EL].bitcast(fp8)
                                          .rearrange("p (two c) -> p two c", two=2),
                                    start=False, stop=(done == nchunks),
                                    perf_mode=DR)
                                j += 2
                            else:
                                if j < KSLOT:
                                    S_ap = sfx_t[:, j * P:(j + 1) * P]
                                else:
                                    S_ap = ssp_t[:, qcol * P:(qcol + 1) * P]
                                    qcol += 1
                                done += 1
                                nc.tensor.matmul(
                                    out=ps[:], lhsT=S_ap,
                                    rhs=gt[:, sl * EL:(sl + 1) * EL].bitcast(fp8)[:, 0:C],
                                    start=False, stop=(done == nchunks))
                                j += 1
                    dv = dscale[:, b:b + 1]
                    if with_b:
                        t1 = spool.tile([P, C], f32, name="t1_t")
                        nc.vector.tensor_scalar(
                            out=t1[:], in0=ps[:], scalar1=dv,
                            scalar2=None, op0=A.mult)
                        t2 = spool.tile([P, C], f32, name="t2_t")
                        nc.vector.tensor_tensor(out=t2[:], in0=t1[:],
                                                in1=bb[:, :C], op=A.add)
                        nc.scalar.activation(out=hsh[:, b * C:(b + 1) * C],
                                             in_=t2[:], func=AF.Relu)
                    elif b % 2 == 0:
                        nc.scalar.activation(out=hsh[:, b * C:(b + 1) * C],
                                             in_=ps[:], func=AF.Relu,
                                             scale=dv)
                    else:
                        nc.vector.tensor_scalar(
                            out=hsh[:, b * C:(b + 1) * C], in0=ps[:],
                            scalar1=dv, scalar2=0.0, op0=A.mult, op1=A.max)
                    if post_block is not None:
                        post_block(b)

            # stage C per block; with b1==0, h1_sh holds relu(dinv^2*agg) so
            # y2 = h1x @ W2 directly (no post-scale)
            def stage_c(b):
                tp = psT.tile([HID, P], bf16, name="psT_t", tag="tp")
                nc.tensor.transpose(out=tp[:], in_=h1_sh[:, b * HID:(b + 1) * HID],
                                    identity=idbf_t[:])
                h1Tb = spool.tile([HID, P], bf16, name="h1Tb_t")
                if b % 2 == 0:
                    nc.vector.tensor_copy(out=h1Tb[:], in_=tp[:])
                else:
                    nc.scalar.activation(out=h1Tb[:], in_=tp[:], func=AF.Copy)
                ps2 = ps2p.tile([P, OUT], f32, name="ps2_t")
                nc.tensor.matmul(out=ps2[:], lhsT=h1Tb[:],
                                 rhs=w2t[:], start=True, stop=True)
                if with_b1:
                    nc.vector.tensor_tensor(
                        out=y2_sh[:, b * OUT:(b + 1) * OUT], in0=ps2[:],
                        in1=dr8[:, b * OUT:(b + 1) * OUT], op=A.mult)
                elif b % 2 == 0:
                    nc.scalar.activation(out=y2_sh[:, b * OUT:(b + 1) * OUT],
                                         in_=ps2[:], func=AF.Copy)
                else:
                    nc.vector.tensor_copy(out=y2_sh[:, b * OUT:(b + 1) * OUT],
                                          in_=ps2[:])
                if b == 7 * SLAB - 1:
                    write_y2cc(0)

            def write_y2cc(piece):
                b0 = 0 if piece == 0 else 7 * SLAB
                b1_ = 7 * SLAB if piece == 0 else bpc
                nc.sync.dma_start(
                    out=y2cc[b0 * P:b1_ * P, :].rearrange("(b p) c -> p b c", p=P),
                    in_=y2_sh[:, b0 * OUT:b1_ * OUT].rearrange(
                        "p (b c) -> p b c", c=OUT))

            agg_layer(y1tab, y1_sh, h1_sh, HID, 8, "ga", with_b1,
                      b1b if with_b1 else None, stage_c, dinvl1_t)
            write_y2cc(1)

            # ---- replicate layer-2 table ----
            _collective_raw(nc.gpsimd, "AllGather", A.bypass,
                            y2cc[:, :], y2win[:, :], [list(range(NCORES))])
            nc.scalar.dma_start(out=y2tab[0:TH, 0:OUT], in_=y2win[0:TH, :])
            nc.scalar.dma_start(out=y2tab[TH:TROWS, 0:OUT], in_=y2win[TH:TROWS, :])

            # layer 2 + pooling per block
            pp = psE.tile([NG, OUT], f32)

            def pool_block(b):
                nc.tensor.matmul(out=pp[:], lhsT=sb_t[:, b * NG:(b + 1) * NG],
                                 rhs=h2_sh[:, b * OUT:(b + 1) * OUT],
                                 start=(b == 0), stop=(b == bpc - 1))

            agg_layer(y2tab, y2_sh, h2_sh, OUT, 2, "gb", with_b2,
                      b2b if with_b2 else None, pool_block, dinv_t)

            sums = cpool.tile([NG, OUT], f32)
            nc.vector.tensor_copy(out=sums[:], in_=pp[:])
            nc.sync.dma_start(out=out[:, :], in_=sums[:])
            if DBG:
                TH2 = TROWS // 2
                nc.gpsimd.dma_start(out=windump[0:TH2, :], in_=y1win[0:TH2, :])
                nc.gpsimd.dma_start(out=windump[TH2:TROWS, :], in_=y1win[TH2:TROWS, :])
                nc.gpsimd.dma_start(out=y1dump[:, :], in_=y1_sh[:])
                nc.gpsimd.dma_start(out=h1dump[:, :], in_=h1_sh[:])
                nc.gpsimd.dma_start(out=y2dump[:, :], in_=y2_sh[:])
                nc.gpsimd.dma_start(out=h2dump[:, :], in_=h2_sh[:])

    nc.compile()
    return nc


def prep_program(x, edge_index, batch, W1, b1, W2, b2):
    """Build the compiled program + per-core input maps + pooling counts."""
    plan, per_core, cnts, xT, sfx, ident8, identbf = host_prep(x, edge_index, batch)
    with_b1 = bool(np.any(np.asarray(b1)))
    with_b2 = bool(np.any(np.asarray(b2)))
    nc = build(plan, with_b1, with_b2)
    W1a = np.asarray(W1, np.float32)
    w1h = np.concatenate([W1a[0:P, :], W1a[P:2 * P, :]], axis=1)  # [P, 2*HID]
    W2a = np.asarray(W2, np.float32)
    w2r = np.zeros((P, OUT), np.float32)
    for j in range(4):
        w2r[j * HID:(j + 1) * HID] = W2a
    in_maps = []
    for c in range(NCORES):
        pc = per_core[c]
        m = dict(
            xT=np.ascontiguousarray(xT[:, c * plan.npc:(c + 1) * plan.npc]),
            idx=pc["idx"], sfx=sfx, ssp=pc["ssp"], ident8=ident8,
            identbf=identbf, W1=w1h, W2=np.asarray(W2, np.float32),
            b1=np.asarray(b1, np.float32).reshape(1, HID),
            b2=np.asarray(b2, np.float32).reshape(1, OUT),
            dinv=pc["dinv"],
            dinv_l1=(pc["dinv"] / 4.0 if with_b1 else pc["dinv_l1"]),
            dinvrep8=pc["dinvrep8"], sb=pc["sb"], w2rep=w2r)
        in_maps.append(m)
    return nc, in_maps, cnts


def run_gnn(x, edge_index, batch, W1, b1, W2, b2):
    from concourse.bass_utils import run_bass_kernel_spmd
    nc, in_maps, cnts = prep_program(x, edge_index, batch, W1, b1, W2, b2)
    res = run_bass_kernel_spmd(nc, in_maps, core_ids=list(range(NCORES)))
    total = np.zeros((NG, OUT), np.float64)
    for c in range(NCORES):
        total += np.asarray(res.results[c]["out"], np.float64)
    pooled = total / np.maximum(cnts, 1.0)[:, None]
    z = pooled - pooled.max(axis=1, keepdims=True)
    ls = z - np.log(np.exp(z).sum(axis=1, keepdims=True))
    return ls.astype(np.float32)


def kernel(x, edge_index, batch, W1, b1, W2, b2):
    """Full-input 2-layer GCN + mean-pool + log_softmax on 8 trn2 NeuronCores."""
    return np.asarray(
        run_gnn(np.asarray(x), np.asarray(edge_index), np.asarray(batch),
                np.asarray(W1), np.asarray(b1), np.asarray(W2), np.asarray(b2)),
        dtype=np.float32)
